# revision 2
# baseline (speedup 1.0000x reference)
"""Trainium2 Bass kernel for the EARLIEST adaptive-halting LSTM.

Shapes (hardcoded from the problem spec):
  x: (T=128, B=512, D=768), u: (T, B), eps: (1,)
  LSTM: H=1024, gates 4H=4096 (torch order i,f,g,o), C=1 output class.

Strategy:
  - Data-parallel over batch: 8 NeuronCores x 64 batch rows each.
  - The halting controller is observational (a/preds/hp never feed back into
    the recurrence), so the device only computes the LSTM scan plus the two
    per-step matvecs h@w_out and h@w_ctrl.  All epsilon-mixing / Bernoulli /
    first-halt logic runs on the host from the tiny (2, T*64) device output.
  - Device layout is "transposed": gates^T with the gate dim on partitions and
    batch on the free dim, so h^T feeds the next step's matmul directly as the
    moving operand with no transposes anywhere in the loop.
  - Phase 1 precomputes Gx = x@W_ih.T + (b_ih+b_hh) for all T in one large
    full-rate matmul pass (removes W_ih from the sequential loop).
  - fp32 everywhere: the u<p halting comparisons are discrete decisions and
    bf16/fp22 matmul error would flip samples.
"""

import sys

sys.path.insert(0, "/opt/trn_rl_repo")

import numpy as np

T_FULL, B, D, H = 128, 512, 768, 1024
NCORES = 8
BL = B // NCORES            # 64 batch rows per core
KD = D // 128               # 6 K-tiles over D
KH = H // 128               # 8 K-tiles over H
M4H = (4 * H) // 128        # 32 M-tiles over the gate dim
FOURH = 4 * H

_compiled = {}


def _build(T):
    import concourse.tile as tile
    from concourse import bacc, mybir

    f32 = mybir.dt.float32
    NCH = (T * BL) // 512
    assert (T * BL) % 512 == 0

    nc = bacc.Bacc("TRN2", target_bir_lowering=False, debug=False,
                   num_devices=NCORES)
    xt_d = nc.dram_tensor("xt", [D, T * BL], f32, kind="ExternalInput")
    wih_d = nc.dram_tensor("wih", [KD, 128, FOURH], f32, kind="ExternalInput")
    whh_d = nc.dram_tensor("whh", [KH, 128, FOURH], f32, kind="ExternalInput")
    wpr_d = nc.dram_tensor("wpr", [KH, 128, 2], f32, kind="ExternalInput")
    bia_d = nc.dram_tensor("bia", [128, M4H], f32, kind="ExternalInput")
    z_d = nc.dram_tensor("z", [2, T * BL], f32, kind="ExternalOutput")

    add = mybir.AluOpType.add
    mult = mybir.AluOpType.mult
    Sig = mybir.ActivationFunctionType.Sigmoid
    Tanh = mybir.ActivationFunctionType.Tanh

    with tile.TileContext(nc) as tc:
        with tc.tile_pool(name="dram", bufs=1, space="DRAM") as dp:
            gx = dp.tile([T, 128, M4H * 64], f32)  # (T, 128, 2048)

            # ---- phase 1: Gx[t] = x_t @ W_ih.T + (b_ih + b_hh), all t ----
            with tc.tile_pool(name="p1w", bufs=1) as p1w, \
                 tc.tile_pool(name="p1x", bufs=12) as p1x, \
                 tc.tile_pool(name="p1s", bufs=6) as p1s, \
                 tc.tile_pool(name="p1p", bufs=8, space="PSUM") as p1p:
                wih_sb = []
                for k in range(KD):
                    w = p1w.tile([128, FOURH], f32, tag=f"wih{k}")
                    nc.sync.dma_start(out=w, in_=wih_d.ap()[k])
                    wih_sb.append(w)
                bias_sb = p1w.tile([128, M4H], f32, tag="bias")
                nc.sync.dma_start(out=bias_sb, in_=bia_d.ap())
                for c in range(NCH):
                    xk = []
                    for k in range(KD):
                        xx = p1x.tile([128, 512], f32, tag="xk")
                        nc.sync.dma_start(
                            out=xx,
                            in_=xt_d.ap()[k * 128:(k + 1) * 128,
                                          c * 512:(c + 1) * 512])
                        xk.append(xx)
                    for m in range(M4H):
                        ps = p1p.tile([128, 512], f32, tag="ps")
                        for k in range(KD):
                            nc.tensor.matmul(ps,
                                             wih_sb[k][:, m * 128:(m + 1) * 128],
                                             xk[k],
                                             start=(k == 0), stop=(k == KD - 1))
                        st = p1s.tile([128, 512], f32, tag="st")
                        nc.vector.tensor_scalar_add(st, ps, bias_sb[:, m:m + 1])
                        dst = gx[8 * c:8 * c + 8, :,
                                 m * 64:(m + 1) * 64].rearrange("t p n -> p t n")
                        nc.sync.dma_start(out=dst, in_=st)

            # ---- phase 2: LSTM scan + [w_out, w_ctrl] matvec per step ----
            with tc.tile_pool(name="p2w", bufs=1) as p2w, \
                 tc.tile_pool(name="p2g", bufs=3) as p2g, \
                 tc.tile_pool(name="p2h", bufs=2) as p2h, \
                 tc.tile_pool(name="p2c", bufs=2) as p2c, \
                 tc.tile_pool(name="p2k", bufs=1) as p2k, \
                 tc.tile_pool(name="p2z", bufs=1) as p2z, \
                 tc.tile_pool(name="p2p", bufs=1, space="PSUM") as p2p, \
                 tc.tile_pool(name="p2q", bufs=2, space="PSUM") as p2q:
                whh_sb = []
                for k in range(KH):
                    w = p2w.tile([128, FOURH], f32, tag=f"whh{k}")
                    nc.sync.dma_start(out=w, in_=whh_d.ap()[k])
                    whh_sb.append(w)
                wpr_sb = []
                for k in range(KH):
                    w = p2w.tile([128, 2], f32, tag=f"wpr{k}")
                    nc.sync.dma_start(out=w, in_=wpr_d.ap()[k])
                    wpr_sb.append(w)
                z_sb = p2z.tile([2, T * BL], f32, tag="zsb")

                h_prev = None
                c_prev = None
                for t in range(T):
                    gxt = p2g.tile([128, M4H * 64], f32, tag="gx")
                    nc.sync.dma_start(out=gxt, in_=gx[t])
                    if t > 0:
                        pzb = [p2p.tile([128, 512], f32, tag=f"pz{g}",
                                        name=f"pz{g}_{t}")
                               for g in range(4)]
                        for m in range(M4H):
                            g, j = divmod(m, 8)
                            for k in range(KH):
                                nc.tensor.matmul(
                                    pzb[g][:, j * 64:(j + 1) * 64],
                                    whh_sb[k][:, m * 128:(m + 1) * 128],
                                    h_prev[:, k * 64:(k + 1) * 64],
                                    start=(k == 0), stop=(k == KH - 1))
                        zi = p2k.tile([128, 512], f32, tag="zi")
                        zf = p2k.tile([128, 512], f32, tag="zf")
                        zg = p2k.tile([128, 512], f32, tag="zg")
                        zo = p2k.tile([128, 512], f32, tag="zo")
                        nc.vector.tensor_tensor(zi, pzb[0], gxt[:, 0:512], add)
                        nc.scalar.activation(zi, zi, Sig)
                        nc.vector.tensor_tensor(zf, pzb[1], gxt[:, 512:1024], add)
                        nc.scalar.activation(zf, zf, Sig)
                        nc.vector.tensor_tensor(zg, pzb[2], gxt[:, 1024:1536], add)
                        nc.scalar.activation(zg, zg, Tanh)
                        nc.vector.tensor_tensor(zo, pzb[3], gxt[:, 1536:2048], add)
                        nc.scalar.activation(zo, zo, Sig)
                        nc.vector.tensor_tensor(zg, zi, zg, mult)       # si*tg
                        nc.vector.tensor_tensor(zf, zf, c_prev, mult)   # sf*c
                        c_new = p2c.tile([128, 512], f32, tag="c")
                        nc.vector.tensor_tensor(c_new, zf, zg, add)
                    else:
                        zi = p2k.tile([128, 512], f32, tag="zi")
                        zg = p2k.tile([128, 512], f32, tag="zg")
                        zo = p2k.tile([128, 512], f32, tag="zo")
                        nc.scalar.activation(zi, gxt[:, 0:512], Sig)
                        nc.scalar.activation(zg, gxt[:, 1024:1536], Tanh)
                        nc.scalar.activation(zo, gxt[:, 1536:2048], Sig)
                        c_new = p2c.tile([128, 512], f32, tag="c")
                        nc.vector.tensor_tensor(c_new, zi, zg, mult)
                    nc.scalar.activation(zi, c_new, Tanh)
                    h_new = p2h.tile([128, 512], f32, tag="h")
                    nc.vector.tensor_tensor(h_new, zo, zi, mult)
                    pzv = p2q.tile([2, 64], f32, tag="pzv")
                    for k in range(KH):
                        nc.tensor.matmul(pzv, wpr_sb[k],
                                         h_new[:, k * 64:(k + 1) * 64],
                                         start=(k == 0), stop=(k == KH - 1))
                    nc.vector.tensor_copy(z_sb[:, t * 64:(t + 1) * 64], pzv)
                    h_prev, c_prev = h_new, c_new
                nc.sync.dma_start(out=z_d.ap(), in_=z_sb)

    nc.compile()
    return nc


def _get_nc(T):
    if T not in _compiled:
        _compiled[T] = _build(T)
    return _compiled[T]


def _prep_inputs(x, W_ih, W_hh, b_ih, b_hh, W_out, W_ctrl):
    T = x.shape[0]
    wih = np.ascontiguousarray(W_ih.T).reshape(KD, 128, FOURH)
    whh = np.ascontiguousarray(W_hh.T).reshape(KH, 128, FOURH)
    wpr = np.ascontiguousarray(
        np.stack([W_out[0], W_ctrl[0, :H]], axis=1)).reshape(KH, 128, 2)
    bia = np.ascontiguousarray((b_ih + b_hh).reshape(M4H, 128).T)
    in_maps = []
    for r in range(NCORES):
        xt = np.ascontiguousarray(
            x[:, r * BL:(r + 1) * BL, :].transpose(2, 0, 1).reshape(D, T * BL))
        in_maps.append({"xt": xt, "wih": wih, "whh": whh, "wpr": wpr,
                        "bia": bia})
    return in_maps


def run_device(x, W_ih, W_hh, b_ih, b_hh, W_out, W_ctrl, trace=False):
    """Run the device part; returns Z (2, T, B) fp32 [h@w_out ; h@w_ctrl_h],
    plus the BassKernelResults (for profiling)."""
    from concourse.bass_utils import run_bass_kernel_spmd

    T = x.shape[0]
    nc = _get_nc(T)
    in_maps = _prep_inputs(x, W_ih, W_hh, b_ih, b_hh, W_out, W_ctrl)
    res = run_bass_kernel_spmd(nc, in_maps, list(range(NCORES)), trace=trace)
    Z = np.empty((2, T, B), np.float32)
    for r in range(NCORES):
        Z[:, :, r * BL:(r + 1) * BL] = res.results[r]["z"].reshape(2, T, BL)
    return Z, res


def _postprocess(Z, u, eps, b_out, W_ctrl, b_ctrl):
    T = Z.shape[1]
    e = np.float64(np.float32(eps[0]))
    logits_all = Z[0] + np.float32(b_out[0])            # (T, B) fp32
    wt = np.float64(W_ctrl[0, H])
    bc = np.float64(b_ctrl[0])
    ts_col = np.arange(T, dtype=np.float64)[:, None]
    zc = Z[1].astype(np.float64) + ts_col * wt + bc
    p = 1.0 / (1.0 + np.exp(-zc))
    p = (1.0 - e) * p + e * 0.05
    p = np.where(np.isclose(p, 0.0), p + 1e-6, p)
    a = u.astype(np.float64) < p                        # (T, B) bool
    Bn = Z.shape[2]
    preds = np.zeros(Bn, np.float64)
    hp = np.full(Bn, -1.0, np.float64)
    for t in range(T):
        halt = a[t]
        upd = halt & (preds == 0.0)
        preds = np.where(upd, logits_all[t].astype(np.float64), preds)
        hpu = (hp == -1.0) & halt
        hp = np.where(hpu, float(t), hp)
    final_logits = logits_all[T - 1].astype(np.float64)
    logits_out = np.where(preds == 0.0, final_logits, preds).astype(np.float32)
    hp2 = np.where(hp == -1.0, float(T - 1), hp)
    halting_points = (hp2 + 1.0).astype(np.float32)
    hmean = np.float32(np.mean(1.0 + hp2) / np.float64(T + 1))
    return logits_out, halting_points, hmean


def kernel(x, u, eps, W_ih, W_hh, b_ih, b_hh, W_out, b_out, W_ctrl, b_ctrl,
           W_base, b_base):
    x = np.asarray(x, np.float32)
    u = np.asarray(u, np.float32)
    Z, _ = run_device(x, np.asarray(W_ih, np.float32),
                      np.asarray(W_hh, np.float32),
                      np.asarray(b_ih, np.float32),
                      np.asarray(b_hh, np.float32),
                      np.asarray(W_out, np.float32),
                      np.asarray(W_ctrl, np.float32))
    return _postprocess(Z, u, np.asarray(eps), np.asarray(b_out),
                        np.asarray(W_ctrl), np.asarray(b_ctrl))


# revision 3
# speedup vs baseline: 1.5497x; 1.5497x over previous
"""Trainium2 Bass kernel for the EARLIEST adaptive-halting LSTM.

Shapes (hardcoded from the problem spec):
  x: (T=128, B=512, D=768), u: (T, B), eps: (1,)
  LSTM: H=1024, gates 4H=4096 (torch order i,f,g,o), C=1 output class.

Strategy:
  - Data-parallel over batch: 8 NeuronCores x 64 batch rows each.
  - The halting controller is observational (a/preds/hp never feed back into
    the recurrence), so the device only computes the LSTM scan plus the two
    per-step matvecs h@w_out and h@w_ctrl.  All epsilon-mixing / Bernoulli /
    first-halt logic runs on the host from the tiny (2, T*64) device output.
  - fp32 matmuls lower to 2 half-speed HW passes with per-pass LDWEIGHTS, so
    weight-stationary layouts with a 64-wide moving operand are
    LDWEIGHTS-bound.  Both phases therefore keep the *small* operand
    stationary and stream the 4H-wide weight matrix as the moving operand
    (N=512 chunks, stream-bound):
      phase 1: stationary X^T blocks (M=128 of T*B), moving W_ih^T -> Gx in
               (t*b, 4H) layout at full rate.
      phase 2: stationary h^T tiles (M=64 batch), moving W_hh^T.  Gates come
               out as (B, 4H); a per-step PE-transpose pass rebuilds h^T.
  - fp32 everywhere: the u<p halting comparisons are discrete decisions and
    bf16/fp22 matmul error would flip samples.
"""

import sys

sys.path.insert(0, "/opt/trn_rl_repo")

import numpy as np

T_FULL, B, D, H = 128, 512, 768, 1024
NCORES = 8
BL = B // NCORES            # 64 batch rows per core
KD = D // 128               # 6 K-tiles over D
KH = H // 128               # 8 K-tiles over H
FOURH = 4 * H

_compiled = {}


def _build(T):
    import concourse.bass as bass
    import concourse.tile as tile
    from concourse import bacc, mybir
    from concourse.masks import make_identity

    f32 = mybir.dt.float32
    NTB = (T * BL) // 128   # number of 128-row blocks of the T*B axis
    assert (T * BL) % 128 == 0

    nc = bacc.Bacc("TRN2", target_bir_lowering=False, debug=False,
                   num_devices=NCORES)
    xt_d = nc.dram_tensor("xt", [D, T * BL], f32, kind="ExternalInput")
    wih_d = nc.dram_tensor("wih", [KD, 128, FOURH], f32, kind="ExternalInput")
    whh_d = nc.dram_tensor("whh", [KH, 128, FOURH], f32, kind="ExternalInput")
    wpr_d = nc.dram_tensor("wpr", [KH, 128, 2], f32, kind="ExternalInput")
    bia_d = nc.dram_tensor("bia", [FOURH], f32, kind="ExternalInput")
    z_d = nc.dram_tensor("z", [2, T * BL], f32, kind="ExternalOutput")

    add = mybir.AluOpType.add
    mult = mybir.AluOpType.mult
    Sig = mybir.ActivationFunctionType.Sigmoid
    Tanh = mybir.ActivationFunctionType.Tanh

    with tile.TileContext(nc) as tc:
        with tc.tile_pool(name="dram", bufs=1, space="DRAM") as dp:
            gx = dp.tile([T, BL, FOURH], f32)

            # ---- phase 1: Gx[t,b,:] = x_t @ W_ih.T + (b_ih + b_hh) ----
            with tc.tile_pool(name="p1w", bufs=1) as p1w, \
                 tc.tile_pool(name="p1x", bufs=18) as p1x, \
                 tc.tile_pool(name="p1s", bufs=6) as p1s, \
                 tc.tile_pool(name="p1p", bufs=8, space="PSUM") as p1p:
                wih_sb = []
                for k in range(KD):
                    w = p1w.tile([128, FOURH], f32, tag=f"wih{k}",
                                 name=f"wih{k}")
                    nc.sync.dma_start(out=w, in_=wih_d.ap()[k])
                    wih_sb.append(w)
                biasb = p1w.tile([128, FOURH], f32, tag="biasb")
                bsrc = bass.AP(tensor=bia_d.ap().tensor, offset=0,
                               ap=[[0, 128], [1, FOURH]])
                nc.sync.dma_start(out=biasb, in_=bsrc)
                for tbi in range(NTB):
                    xst = []
                    for k in range(KD):
                        xx = p1x.tile([128, 128], f32, tag="xst",
                                      name=f"xst{tbi}_{k}")
                        nc.sync.dma_start(
                            out=xx,
                            in_=xt_d.ap()[k * 128:(k + 1) * 128,
                                          tbi * 128:(tbi + 1) * 128])
                        xst.append(xx)
                    for c8 in range(8):
                        cs = slice(c8 * 512, (c8 + 1) * 512)
                        ps = p1p.tile([128, 512], f32, tag="ps",
                                      name=f"ps{tbi}_{c8}")
                        for k in range(KD):
                            nc.tensor.matmul(ps, xst[k], wih_sb[k][:, cs],
                                             start=(k == 0), stop=(k == KD - 1))
                        st = p1s.tile([128, 512], f32, tag="st",
                                      name=f"st{tbi}_{c8}")
                        nc.vector.tensor_tensor(st, ps, biasb[:, cs], add)
                        dst = gx[2 * tbi:2 * tbi + 2, :, cs].rearrange(
                            "t b n -> (t b) n")
                        nc.sync.dma_start(out=dst, in_=st)

            # ---- phase 2: LSTM scan + [w_out, w_ctrl] matvec per step ----
            with tc.tile_pool(name="p2w", bufs=1) as p2w, \
                 tc.tile_pool(name="p2g", bufs=2) as p2g, \
                 tc.tile_pool(name="p2h", bufs=2) as p2h, \
                 tc.tile_pool(name="p2c", bufs=2) as p2c, \
                 tc.tile_pool(name="p2k", bufs=2) as p2k, \
                 tc.tile_pool(name="p2t", bufs=2) as p2t, \
                 tc.tile_pool(name="p2p", bufs=6, space="PSUM") as p2p, \
                 tc.tile_pool(name="p2r", bufs=1, space="PSUM") as p2r, \
                 tc.tile_pool(name="p2q", bufs=1, space="PSUM") as p2q:
                whh_sb = []
                for k in range(KH):
                    w = p2w.tile([128, FOURH], f32, tag=f"whh{k}",
                                 name=f"whh{k}")
                    nc.sync.dma_start(out=w, in_=whh_d.ap()[k])
                    whh_sb.append(w)
                wpr_sb = []
                for k in range(KH):
                    w = p2w.tile([128, 2], f32, tag=f"wpr{k}", name=f"wpr{k}")
                    nc.sync.dma_start(out=w, in_=wpr_d.ap()[k])
                    wpr_sb.append(w)
                ident = p2w.tile([128, 128], f32, tag="ident")
                make_identity(nc, ident)

                hT_prev = None
                c_prev = None
                for t in range(T):
                    gxt = p2g.tile([BL, FOURH], f32, tag="gx", name=f"gx{t}")
                    nc.sync.dma_start(out=gxt, in_=gx[t])
                    c_new = p2c.tile([BL, H], f32, tag="c", name=f"c{t}")
                    h_new = p2h.tile([BL, H], f32, tag="h", name=f"h{t}")
                    # chunk (g, hh): gate g, h-half hh; emitted hh-major so
                    # half-0 finishes early and overlaps half-1 matmuls.
                    for hh in range(2):
                        zt = {}
                        for g in range(4):
                            blk = slice(1024 * g + 512 * hh,
                                        1024 * g + 512 * hh + 512)
                            if t > 0:
                                pg = p2p.tile([BL, 512], f32, tag="gp",
                                              name=f"gp{t}_{g}_{hh}")
                                for k in range(KH):
                                    nc.tensor.matmul(
                                        pg,
                                        hT_prev[:, k * 64:(k + 1) * 64],
                                        whh_sb[k][:, blk],
                                        start=(k == 0), stop=(k == KH - 1))
                                z = p2k.tile([BL, 512], f32, tag=f"z{g}",
                                             name=f"z{t}_{g}_{hh}")
                                nc.vector.tensor_tensor(z, pg, gxt[:, blk],
                                                        add)
                            else:
                                z = p2k.tile([BL, 512], f32, tag=f"z{g}",
                                             name=f"z{t}_{g}_{hh}")
                                nc.vector.tensor_copy(z, gxt[:, blk])
                            nc.scalar.activation(
                                z, z, Tanh if g == 2 else Sig)
                            zt[g] = z
                        hs = slice(512 * hh, 512 * hh + 512)
                        # c = sig(f)*c_prev + sig(i)*tanh(g)
                        nc.vector.tensor_tensor(zt[2], zt[0], zt[2], mult)
                        if t > 0:
                            nc.vector.tensor_tensor(zt[1], zt[1],
                                                    c_prev[:, hs], mult)
                            nc.vector.tensor_tensor(c_new[:, hs], zt[1],
                                                    zt[2], add)
                        else:
                            nc.vector.tensor_copy(c_new[:, hs], zt[2])
                        nc.scalar.activation(zt[0], c_new[:, hs], Tanh)
                        nc.vector.tensor_tensor(h_new[:, hs], zt[3], zt[0],
                                                mult)
                    # h^T via PE transpose, all 8 tiles into one PSUM bank
                    pht = p2r.tile([128, 512], f32, tag="pht", name=f"pht{t}")
                    for k in range(KH):
                        nc.tensor.transpose(
                            pht[:, k * 64:(k + 1) * 64],
                            h_new[:, k * 128:(k + 1) * 128],
                            ident[0:BL, 0:BL])
                    hT_new = p2t.tile([128, 512], f32, tag="hT",
                                      name=f"hT{t}")
                    nc.vector.tensor_copy(hT_new, pht)
                    pzv = p2q.tile([2, 64], f32, tag="pzv", name=f"pzv{t}")
                    for k in range(KH):
                        nc.tensor.matmul(pzv, wpr_sb[k],
                                         hT_new[:, k * 64:(k + 1) * 64],
                                         start=(k == 0), stop=(k == KH - 1))
                    zz = p2t.tile([2, 64], f32, tag="zz", name=f"zz{t}")
                    nc.vector.tensor_copy(zz, pzv)
                    nc.sync.dma_start(out=z_d.ap()[:, t * 64:(t + 1) * 64],
                                      in_=zz)
                    hT_prev, c_prev = hT_new, c_new

    nc.compile()
    return nc


def _get_nc(T):
    if T not in _compiled:
        _compiled[T] = _build(T)
    return _compiled[T]


def _prep_inputs(x, W_ih, W_hh, b_ih, b_hh, W_out, W_ctrl):
    T = x.shape[0]
    wih = np.ascontiguousarray(W_ih.T).reshape(KD, 128, FOURH)
    whh = np.ascontiguousarray(W_hh.T).reshape(KH, 128, FOURH)
    wpr = np.ascontiguousarray(
        np.stack([W_out[0], W_ctrl[0, :H]], axis=1)).reshape(KH, 128, 2)
    bia = np.ascontiguousarray(b_ih + b_hh)
    in_maps = []
    for r in range(NCORES):
        xt = np.ascontiguousarray(
            x[:, r * BL:(r + 1) * BL, :].transpose(2, 0, 1).reshape(D, T * BL))
        in_maps.append({"xt": xt, "wih": wih, "whh": whh, "wpr": wpr,
                        "bia": bia})
    return in_maps


def run_device(x, W_ih, W_hh, b_ih, b_hh, W_out, W_ctrl, trace=False):
    """Run the device part; returns Z (2, T, B) fp32 [h@w_out ; h@w_ctrl_h],
    plus the BassKernelResults (for profiling)."""
    from concourse.bass_utils import run_bass_kernel_spmd

    T = x.shape[0]
    nc = _get_nc(T)
    in_maps = _prep_inputs(x, W_ih, W_hh, b_ih, b_hh, W_out, W_ctrl)
    res = run_bass_kernel_spmd(nc, in_maps, list(range(NCORES)), trace=trace)
    Z = np.empty((2, T, B), np.float32)
    for r in range(NCORES):
        Z[:, :, r * BL:(r + 1) * BL] = res.results[r]["z"].reshape(2, T, BL)
    return Z, res


def _postprocess(Z, u, eps, b_out, W_ctrl, b_ctrl):
    T = Z.shape[1]
    e = np.float64(np.float32(eps[0]))
    logits_all = Z[0] + np.float32(b_out[0])            # (T, B) fp32
    wt = np.float64(W_ctrl[0, H])
    bc = np.float64(b_ctrl[0])
    ts_col = np.arange(T, dtype=np.float64)[:, None]
    zc = Z[1].astype(np.float64) + ts_col * wt + bc
    p = 1.0 / (1.0 + np.exp(-zc))
    p = (1.0 - e) * p + e * 0.05
    p = np.where(np.isclose(p, 0.0), p + 1e-6, p)
    a = u.astype(np.float64) < p                        # (T, B) bool
    Bn = Z.shape[2]
    preds = np.zeros(Bn, np.float64)
    hp = np.full(Bn, -1.0, np.float64)
    for t in range(T):
        halt = a[t]
        upd = halt & (preds == 0.0)
        preds = np.where(upd, logits_all[t].astype(np.float64), preds)
        hpu = (hp == -1.0) & halt
        hp = np.where(hpu, float(t), hp)
    final_logits = logits_all[T - 1].astype(np.float64)
    logits_out = np.where(preds == 0.0, final_logits, preds).astype(np.float32)
    hp2 = np.where(hp == -1.0, float(T - 1), hp)
    halting_points = (hp2 + 1.0).astype(np.float32)
    hmean = np.float32(np.mean(1.0 + hp2) / np.float64(T + 1))
    return logits_out, halting_points, hmean


def kernel(x, u, eps, W_ih, W_hh, b_ih, b_hh, W_out, b_out, W_ctrl, b_ctrl,
           W_base, b_base):
    x = np.asarray(x, np.float32)
    u = np.asarray(u, np.float32)
    Z, _ = run_device(x, np.asarray(W_ih, np.float32),
                      np.asarray(W_hh, np.float32),
                      np.asarray(b_ih, np.float32),
                      np.asarray(b_hh, np.float32),
                      np.asarray(W_out, np.float32),
                      np.asarray(W_ctrl, np.float32))
    return _postprocess(Z, u, np.asarray(eps), np.asarray(b_out),
                        np.asarray(W_ctrl), np.asarray(b_ctrl))


# revision 4
# speedup vs baseline: 2.0079x; 1.2957x over previous
"""Trainium2 Bass kernel for the EARLIEST adaptive-halting LSTM.

Shapes (hardcoded from the problem spec):
  x: (T=128, B=512, D=768), u: (T, B), eps: (1,)
  LSTM: H=1024, gates 4H=4096 (torch order i,f,g,o), C=1 output class.

Strategy:
  - Data-parallel over batch: 8 NeuronCores x 64 batch rows each.
  - The halting controller is observational (a/preds/hp never feed back into
    the recurrence), so the device only computes the LSTM scan plus the two
    per-step matvecs h@w_out and h@w_ctrl.  All epsilon-mixing / Bernoulli /
    first-halt logic runs on the host from the tiny (2, T*64) device output.
  - fp32 matmuls run at 4 cycles/row (2 half-speed HW passes, each with its
    own LDWEIGHTS).  Instead, weights and activations are split into fp16
    hi/lo pairs (W = Wh + Wl, h = hh + hl, each fp16 = 10+ mantissa bits, so
    the pair carries ~21 bits) and each matmul is computed as three
    full-rate fp16 passes Wh*hh + Wh*hl + Wl*hh accumulated in fp32 PSUM.
    That is 3 cycles/row of fp16 work vs 4 for native fp32, with ~fp32
    accuracy (validated against the reference: halting decisions exact).
  - Both phases keep the *small* operand stationary and stream the 4H-wide
    weight matrix as the moving operand (N=512 chunks, stream-bound):
      phase 1: stationary X^T blocks (M=128 of T*B), moving W_ih^T -> Gx in
               (t*b, 4H) layout at full rate.
      phase 2: stationary h^T tiles (M=64 batch), moving W_hh^T.  Gates come
               out as (B, 4H); a per-step PE-transpose pass rebuilds h^T.
"""

import sys

sys.path.insert(0, "/opt/trn_rl_repo")

import numpy as np

T_FULL, B, D, H = 128, 512, 768, 1024
NCORES = 8
BL = B // NCORES            # 64 batch rows per core
KD = D // 128               # 6 K-tiles over D
KH = H // 128               # 8 K-tiles over H
FOURH = 4 * H

_compiled = {}


def _build(T):
    import concourse.bass as bass
    import concourse.tile as tile
    from concourse import bacc, mybir
    from concourse.masks import make_identity

    f32 = mybir.dt.float32
    f16 = mybir.dt.float16
    NTB = (T * BL) // 128   # number of 128-row blocks of the T*B axis
    assert (T * BL) % 128 == 0

    nc = bacc.Bacc("TRN2", target_bir_lowering=False, debug=False,
                   num_devices=NCORES)
    xt_d = nc.dram_tensor("xt", [D, T * BL], f32, kind="ExternalInput")
    wihh_d = nc.dram_tensor("wihh", [KD, 128, FOURH], f16,
                            kind="ExternalInput")
    wihl_d = nc.dram_tensor("wihl", [KD, 128, FOURH], f16,
                            kind="ExternalInput")
    whhh_d = nc.dram_tensor("whhh", [KH, 128, FOURH], f16,
                            kind="ExternalInput")
    whhl_d = nc.dram_tensor("whhl", [KH, 128, FOURH], f16,
                            kind="ExternalInput")
    wpr_d = nc.dram_tensor("wpr", [KH, 128, 2], f32, kind="ExternalInput")
    bia_d = nc.dram_tensor("bia", [FOURH], f32, kind="ExternalInput")
    z_d = nc.dram_tensor("z", [2, T * BL], f32, kind="ExternalOutput")

    add = mybir.AluOpType.add
    sub = mybir.AluOpType.subtract
    mult = mybir.AluOpType.mult
    Sig = mybir.ActivationFunctionType.Sigmoid
    Tanh = mybir.ActivationFunctionType.Tanh

    with tile.TileContext(nc) as tc:
        with tc.tile_pool(name="dram", bufs=1, space="DRAM") as dp:
            gx = dp.tile([T, BL, FOURH], f32)

            # ---- phase 1: Gx[t,b,:] = x_t @ W_ih.T + (b_ih + b_hh) ----
            with tc.tile_pool(name="p1w", bufs=1) as p1w, \
                 tc.tile_pool(name="p1x", bufs=18) as p1x, \
                 tc.tile_pool(name="p1s", bufs=6) as p1s, \
                 tc.tile_pool(name="p1p", bufs=8, space="PSUM") as p1p:
                wih_sb = []
                for k in range(KD):
                    wh = p1w.tile([128, FOURH], f16, tag=f"wihh{k}",
                                  name=f"wihh{k}")
                    nc.sync.dma_start(out=wh, in_=wihh_d.ap()[k])
                    wl = p1w.tile([128, FOURH], f16, tag=f"wihl{k}",
                                  name=f"wihl{k}")
                    nc.sync.dma_start(out=wl, in_=wihl_d.ap()[k])
                    wih_sb.append((wh, wl))
                biasb = p1w.tile([128, FOURH], f32, tag="biasb")
                bsrc = bass.AP(tensor=bia_d.ap().tensor, offset=0,
                               ap=[[0, 128], [1, FOURH]])
                nc.sync.dma_start(out=biasb, in_=bsrc)
                for tbi in range(NTB):
                    xst = []
                    for k in range(KD):
                        xx = p1x.tile([128, 128], f32, tag="xst",
                                      name=f"xst{tbi}_{k}")
                        nc.sync.dma_start(
                            out=xx,
                            in_=xt_d.ap()[k * 128:(k + 1) * 128,
                                          tbi * 128:(tbi + 1) * 128])
                        xh = p1x.tile([128, 128], f16, tag="xsth",
                                      name=f"xsth{tbi}_{k}")
                        nc.vector.tensor_copy(xh, xx)
                        xl = p1x.tile([128, 128], f16, tag="xstl",
                                      name=f"xstl{tbi}_{k}")
                        nc.vector.tensor_tensor(xl, xx, xh, sub)
                        xst.append((xh, xl))
                    for c8 in range(8):
                        cs = slice(c8 * 512, (c8 + 1) * 512)
                        ps = p1p.tile([128, 512], f32, tag="ps",
                                      name=f"ps{tbi}_{c8}")
                        nmm = 3 * KD
                        i = 0
                        for k in range(KD):
                            xh, xl = xst[k]
                            wh, wl = wih_sb[k]
                            for lhsT, rhs in ((xh, wh), (xl, wh), (xh, wl)):
                                nc.tensor.matmul(ps, lhsT, rhs[:, cs],
                                                 start=(i == 0),
                                                 stop=(i == nmm - 1))
                                i += 1
                        st = p1s.tile([128, 512], f32, tag="st",
                                      name=f"st{tbi}_{c8}")
                        nc.vector.tensor_tensor(st, ps, biasb[:, cs], add)
                        dst = gx[2 * tbi:2 * tbi + 2, :, cs].rearrange(
                            "t b n -> (t b) n")
                        nc.sync.dma_start(out=dst, in_=st)

            # ---- phase 2: LSTM scan + [w_out, w_ctrl] matvec per step ----
            with tc.tile_pool(name="p2w", bufs=1) as p2w, \
                 tc.tile_pool(name="p2g", bufs=2) as p2g, \
                 tc.tile_pool(name="p2h", bufs=2) as p2h, \
                 tc.tile_pool(name="p2c", bufs=2) as p2c, \
                 tc.tile_pool(name="p2k", bufs=2) as p2k, \
                 tc.tile_pool(name="p2t", bufs=2) as p2t, \
                 tc.tile_pool(name="p2p", bufs=6, space="PSUM") as p2p, \
                 tc.tile_pool(name="p2r", bufs=1, space="PSUM") as p2r, \
                 tc.tile_pool(name="p2q", bufs=1, space="PSUM") as p2q:
                whh_sb = []
                for k in range(KH):
                    wh = p2w.tile([128, FOURH], f16, tag=f"whhh{k}",
                                  name=f"whhh{k}")
                    nc.sync.dma_start(out=wh, in_=whhh_d.ap()[k])
                    wl = p2w.tile([128, FOURH], f16, tag=f"whhl{k}",
                                  name=f"whhl{k}")
                    nc.sync.dma_start(out=wl, in_=whhl_d.ap()[k])
                    whh_sb.append((wh, wl))
                wpr_sb = []
                for k in range(KH):
                    w = p2w.tile([128, 2], f32, tag=f"wpr{k}", name=f"wpr{k}")
                    nc.sync.dma_start(out=w, in_=wpr_d.ap()[k])
                    wpr_sb.append(w)
                ident = p2w.tile([128, 128], f32, tag="ident")
                make_identity(nc, ident)

                hTh_prev = None
                hTl_prev = None
                c_prev = None
                for t in range(T):
                    gxt = p2g.tile([BL, FOURH], f32, tag="gx", name=f"gx{t}")
                    nc.sync.dma_start(out=gxt, in_=gx[t])
                    c_new = p2c.tile([BL, H], f32, tag="c", name=f"c{t}")
                    h_new = p2h.tile([BL, H], f32, tag="h", name=f"h{t}")
                    # chunk (g, hh): gate g, h-half hh; emitted hh-major so
                    # half-0 finishes early and overlaps half-1 matmuls.
                    for hh in range(2):
                        zt = {}
                        for g in range(4):
                            blk = slice(1024 * g + 512 * hh,
                                        1024 * g + 512 * hh + 512)
                            z = p2k.tile([BL, 512], f32, tag=f"z{g}",
                                         name=f"z{t}_{g}_{hh}")
                            if t > 0:
                                pg = p2p.tile([BL, 512], f32, tag="gp",
                                              name=f"gp{t}_{g}_{hh}")
                                i = 0
                                for k in range(KH):
                                    ks = slice(k * 64, (k + 1) * 64)
                                    wh, wl = whh_sb[k]
                                    for lhsT, rhs in ((hTh_prev, wh),
                                                      (hTl_prev, wh),
                                                      (hTh_prev, wl)):
                                        nc.tensor.matmul(
                                            pg, lhsT[:, ks], rhs[:, blk],
                                            start=(i == 0),
                                            stop=(i == 3 * KH - 1))
                                        i += 1
                                nc.vector.tensor_tensor(z, pg, gxt[:, blk],
                                                        add)
                            else:
                                nc.vector.tensor_copy(z, gxt[:, blk])
                            nc.scalar.activation(
                                z, z, Tanh if g == 2 else Sig)
                            zt[g] = z
                        hs = slice(512 * hh, 512 * hh + 512)
                        # c = sig(f)*c_prev + sig(i)*tanh(g)
                        nc.vector.tensor_tensor(zt[2], zt[0], zt[2], mult)
                        if t > 0:
                            nc.vector.tensor_tensor(zt[1], zt[1],
                                                    c_prev[:, hs], mult)
                            nc.vector.tensor_tensor(c_new[:, hs], zt[1],
                                                    zt[2], add)
                        else:
                            nc.vector.tensor_copy(c_new[:, hs], zt[2])
                        nc.scalar.activation(zt[0], c_new[:, hs], Tanh)
                        nc.vector.tensor_tensor(h_new[:, hs], zt[3], zt[0],
                                                mult)
                    # h^T via PE transpose, all 8 tiles into one PSUM bank
                    pht = p2r.tile([128, 512], f32, tag="pht", name=f"pht{t}")
                    for k in range(KH):
                        nc.tensor.transpose(
                            pht[:, k * 64:(k + 1) * 64],
                            h_new[:, k * 128:(k + 1) * 128],
                            ident[0:BL, 0:BL])
                    hT_new = p2t.tile([128, 512], f32, tag="hT",
                                      name=f"hT{t}")
                    nc.vector.tensor_copy(hT_new, pht)
                    hTh = p2t.tile([128, 512], f16, tag="hTh",
                                   name=f"hTh{t}")
                    nc.vector.tensor_copy(hTh, hT_new)
                    hTl = p2t.tile([128, 512], f16, tag="hTl",
                                   name=f"hTl{t}")
                    nc.vector.tensor_tensor(hTl, hT_new, hTh, sub)
                    pzv = p2q.tile([2, 64], f32, tag="pzv", name=f"pzv{t}")
                    for k in range(KH):
                        nc.tensor.matmul(pzv, wpr_sb[k],
                                         hT_new[:, k * 64:(k + 1) * 64],
                                         start=(k == 0), stop=(k == KH - 1))
                    zz = p2t.tile([2, 64], f32, tag="zz", name=f"zz{t}")
                    nc.vector.tensor_copy(zz, pzv)
                    nc.sync.dma_start(out=z_d.ap()[:, t * 64:(t + 1) * 64],
                                      in_=zz)
                    hTh_prev, hTl_prev, c_prev = hTh, hTl, c_new

    nc.compile()
    return nc


def _get_nc(T):
    if T not in _compiled:
        _compiled[T] = _build(T)
    return _compiled[T]


def _split16(w):
    """Split fp32 matrix into fp16 hi/lo pair with hi+lo ~= w (21 bits)."""
    wh = w.astype(np.float16)
    wl = (w - wh.astype(np.float32)).astype(np.float16)
    return wh, wl


def _prep_inputs(x, W_ih, W_hh, b_ih, b_hh, W_out, W_ctrl):
    T = x.shape[0]
    wih = np.ascontiguousarray(W_ih.T).reshape(KD, 128, FOURH)
    whh = np.ascontiguousarray(W_hh.T).reshape(KH, 128, FOURH)
    wihh, wihl = _split16(wih)
    whhh, whhl = _split16(whh)
    wpr = np.ascontiguousarray(
        np.stack([W_out[0], W_ctrl[0, :H]], axis=1)).reshape(KH, 128, 2)
    bia = np.ascontiguousarray(b_ih + b_hh)
    in_maps = []
    for r in range(NCORES):
        xt = np.ascontiguousarray(
            x[:, r * BL:(r + 1) * BL, :].transpose(2, 0, 1).reshape(D, T * BL))
        in_maps.append({"xt": xt, "wihh": wihh, "wihl": wihl,
                        "whhh": whhh, "whhl": whhl, "wpr": wpr, "bia": bia})
    return in_maps


def run_device(x, W_ih, W_hh, b_ih, b_hh, W_out, W_ctrl, trace=False):
    """Run the device part; returns Z (2, T, B) fp32 [h@w_out ; h@w_ctrl_h],
    plus the BassKernelResults (for profiling)."""
    from concourse.bass_utils import run_bass_kernel_spmd

    T = x.shape[0]
    nc = _get_nc(T)
    in_maps = _prep_inputs(x, W_ih, W_hh, b_ih, b_hh, W_out, W_ctrl)
    res = run_bass_kernel_spmd(nc, in_maps, list(range(NCORES)), trace=trace)
    Z = np.empty((2, T, B), np.float32)
    for r in range(NCORES):
        Z[:, :, r * BL:(r + 1) * BL] = res.results[r]["z"].reshape(2, T, BL)
    return Z, res


def _postprocess(Z, u, eps, b_out, W_ctrl, b_ctrl):
    T = Z.shape[1]
    e = np.float64(np.float32(eps[0]))
    logits_all = Z[0] + np.float32(b_out[0])            # (T, B) fp32
    wt = np.float64(W_ctrl[0, H])
    bc = np.float64(b_ctrl[0])
    ts_col = np.arange(T, dtype=np.float64)[:, None]
    zc = Z[1].astype(np.float64) + ts_col * wt + bc
    p = 1.0 / (1.0 + np.exp(-zc))
    p = (1.0 - e) * p + e * 0.05
    p = np.where(np.isclose(p, 0.0), p + 1e-6, p)
    a = u.astype(np.float64) < p                        # (T, B) bool
    Bn = Z.shape[2]
    preds = np.zeros(Bn, np.float64)
    hp = np.full(Bn, -1.0, np.float64)
    for t in range(T):
        halt = a[t]
        upd = halt & (preds == 0.0)
        preds = np.where(upd, logits_all[t].astype(np.float64), preds)
        hpu = (hp == -1.0) & halt
        hp = np.where(hpu, float(t), hp)
    final_logits = logits_all[T - 1].astype(np.float64)
    logits_out = np.where(preds == 0.0, final_logits, preds).astype(np.float32)
    hp2 = np.where(hp == -1.0, float(T - 1), hp)
    halting_points = (hp2 + 1.0).astype(np.float32)
    hmean = np.float32(np.mean(1.0 + hp2) / np.float64(T + 1))
    return logits_out, halting_points, hmean


def kernel(x, u, eps, W_ih, W_hh, b_ih, b_hh, W_out, b_out, W_ctrl, b_ctrl,
           W_base, b_base):
    x = np.asarray(x, np.float32)
    u = np.asarray(u, np.float32)
    Z, _ = run_device(x, np.asarray(W_ih, np.float32),
                      np.asarray(W_hh, np.float32),
                      np.asarray(b_ih, np.float32),
                      np.asarray(b_hh, np.float32),
                      np.asarray(W_out, np.float32),
                      np.asarray(W_ctrl, np.float32))
    return _postprocess(Z, u, np.asarray(eps), np.asarray(b_out),
                        np.asarray(W_ctrl), np.asarray(b_ctrl))


# revision 10
# speedup vs baseline: 2.4305x; 1.2104x over previous
"""Trainium2 Bass kernel for the EARLIEST adaptive-halting LSTM.

Shapes (hardcoded from the problem spec):
  x: (T=128, B=512, D=768), u: (T, B), eps: (1,)
  LSTM: H=1024, gates 4H=4096 (torch order i,f,g,o), C=1 output class.

Strategy:
  - Data-parallel over batch: 8 NeuronCores x 64 batch rows each.
  - The halting controller is observational (a/preds/hp never feed back into
    the recurrence), so the device only computes the LSTM scan plus the two
    per-step matvecs h@w_out and h@w_ctrl.  All epsilon-mixing / Bernoulli /
    first-halt logic runs on the host from the tiny (2, T*64) device output.
  - fp32 matmuls run at 4 cycles/row (2 half-speed HW passes, each with its
    own LDWEIGHTS).  Instead, weights and activations are split into fp16
    hi/lo pairs (W = Wh + Wl, h = hh + hl, each fp16 = 10+ mantissa bits, so
    the pair carries ~21 bits) and each matmul is computed as three
    full-rate fp16 passes Wh*hh + Wh*hl + Wl*hh accumulated in fp32 PSUM.
    That is 3 cycles/row of fp16 work vs 4 for native fp32, with ~fp32
    accuracy (validated against the reference: halting decisions exact).
  - Both phases keep the *small* operand stationary and stream the 4H-wide
    weight matrix as the moving operand (N=512 chunks, stream-bound):
      phase 1: stationary X^T blocks (M=128 of T*B), moving W_ih^T -> Gx in
               (t*b, 4H) layout at full rate.
      phase 2: stationary h^T tiles (M=64 batch), moving W_hh^T.  Gates come
               out as (B, 4H); a per-step PE-transpose pass rebuilds h^T.
"""

import sys

sys.path.insert(0, "/opt/trn_rl_repo")

import numpy as np

T_FULL, B, D, H = 128, 512, 768, 1024
NCORES = 8
BL = B // NCORES            # 64 batch rows per core
KD = D // 128               # 6 K-tiles over D
KH = H // 128               # 8 K-tiles over H
FOURH = 4 * H

_compiled = {}


def _build(T):
    import concourse.bass as bass
    import concourse.tile as tile
    from concourse import bacc, mybir
    from concourse.masks import make_identity

    f32 = mybir.dt.float32
    f16 = mybir.dt.float16
    NTB = (T * BL) // 128   # number of 128-row blocks of the T*B axis
    assert (T * BL) % 128 == 0

    nc = bacc.Bacc("TRN2", target_bir_lowering=False, debug=False,
                   num_devices=NCORES)
    xt_d = nc.dram_tensor("xt", [D, T * BL], f32, kind="ExternalInput")
    wihh_d = nc.dram_tensor("wihh", [KD, 128, FOURH], f16,
                            kind="ExternalInput")
    wihl_d = nc.dram_tensor("wihl", [KD, 128, FOURH], f16,
                            kind="ExternalInput")
    whhh_d = nc.dram_tensor("whhh", [KH, 128, FOURH], f16,
                            kind="ExternalInput")
    whhl_d = nc.dram_tensor("whhl", [KH, 128, FOURH], f16,
                            kind="ExternalInput")
    wpr_d = nc.dram_tensor("wpr", [KH, 128, 2], f32, kind="ExternalInput")
    bia_d = nc.dram_tensor("bia", [FOURH], f32, kind="ExternalInput")
    z_d = nc.dram_tensor("z", [2, T * BL], f32, kind="ExternalOutput")

    add = mybir.AluOpType.add
    sub = mybir.AluOpType.subtract
    mult = mybir.AluOpType.mult
    Sig = mybir.ActivationFunctionType.Sigmoid
    Tanh = mybir.ActivationFunctionType.Tanh

    with tile.TileContext(nc) as tc:
        with tc.tile_pool(name="dram", bufs=1, space="DRAM") as dp:
            # "Folded" gates layout: partition = batch + 64*h_half, free =
            # gate*512 + col.  Lets two M=64 matmuls run concurrently in the
            # PE array via column tiling (tile_position col 0 / 64).
            gx = dp.tile([T, 2 * BL, FOURH // 2], f32)

            # ---- phase 1: Gx[t,b,:] = x_t @ W_ih.T + (b_ih + b_hh) ----
            with tc.tile_pool(name="p1w", bufs=1) as p1w, \
                 tc.tile_pool(name="p1x", bufs=18) as p1x, \
                 tc.tile_pool(name="p1s", bufs=6) as p1s, \
                 tc.tile_pool(name="p1p", bufs=8, space="PSUM") as p1p:
                wih_sb = []
                for k in range(KD):
                    wh = p1w.tile([128, FOURH], f16, tag=f"wihh{k}",
                                  name=f"wihh{k}")
                    nc.sync.dma_start(out=wh, in_=wihh_d.ap()[k])
                    wl = p1w.tile([128, FOURH], f16, tag=f"wihl{k}",
                                  name=f"wihl{k}")
                    nc.sync.dma_start(out=wl, in_=wihl_d.ap()[k])
                    wih_sb.append((wh, wl))
                biasb = p1w.tile([128, FOURH], f32, tag="biasb")
                bsrc = bass.AP(tensor=bia_d.ap().tensor, offset=0,
                               ap=[[0, 128], [1, FOURH]])
                nc.sync.dma_start(out=biasb, in_=bsrc)
                for tbi in range(NTB):
                    xst = []
                    for k in range(KD):
                        xx = p1x.tile([128, 128], f32, tag="xst",
                                      name=f"xst{tbi}_{k}")
                        nc.sync.dma_start(
                            out=xx,
                            in_=xt_d.ap()[k * 128:(k + 1) * 128,
                                          tbi * 128:(tbi + 1) * 128])
                        xh = p1x.tile([128, 128], f16, tag="xsth",
                                      name=f"xsth{tbi}_{k}")
                        nc.vector.tensor_copy(xh, xx)
                        xl = p1x.tile([128, 128], f16, tag="xstl",
                                      name=f"xstl{tbi}_{k}")
                        nc.vector.tensor_tensor(xl, xx, xh, sub)
                        xst.append((xh, xl))
                    for c8 in range(8):
                        cs = slice(c8 * 512, (c8 + 1) * 512)
                        ps = p1p.tile([128, 512], f32, tag="ps",
                                      name=f"ps{tbi}_{c8}")
                        nmm = 3 * KD
                        i = 0
                        for k in range(KD):
                            xh, xl = xst[k]
                            wh, wl = wih_sb[k]
                            for lhsT, rhs in ((xh, wh), (xl, wh), (xh, wl)):
                                nc.tensor.matmul(ps, lhsT, rhs[:, cs],
                                                 start=(i == 0),
                                                 stop=(i == nmm - 1))
                                i += 1
                        st = p1s.tile([128, 512], f32, tag="st",
                                      name=f"st{tbi}_{c8}")
                        nc.vector.tensor_tensor(st, ps, biasb[:, cs], add)
                        g8, hh8 = divmod(c8, 2)
                        dst = gx[2 * tbi:2 * tbi + 2,
                                 64 * hh8:64 * hh8 + 64,
                                 512 * g8:512 * g8 + 512]
                        nc.sync.dma_start(out=dst, in_=st)

            # ---- phase 2: LSTM scan + [w_out, w_ctrl] matvec per step ----
            with tc.tile_pool(name="p2w", bufs=1) as p2w, \
                 tc.tile_pool(name="p2g", bufs=2) as p2g, \
                 tc.tile_pool(name="p2h", bufs=2) as p2h, \
                 tc.tile_pool(name="p2c", bufs=2) as p2c, \
                 tc.tile_pool(name="p2k", bufs=2) as p2k, \
                 tc.tile_pool(name="p2t", bufs=2) as p2t, \
                 tc.tile_pool(name="p2p", bufs=1, space="PSUM") as p2p, \
                 tc.tile_pool(name="p2r", bufs=2, space="PSUM") as p2r, \
                 tc.tile_pool(name="p2q", bufs=2, space="PSUM") as p2q:
                whh_sb = []
                for k in range(KH):
                    wh = p2w.tile([128, FOURH], f16, tag=f"whhh{k}",
                                  name=f"whhh{k}")
                    nc.sync.dma_start(out=wh, in_=whhh_d.ap()[k])
                    wl = p2w.tile([128, FOURH], f16, tag=f"whhl{k}",
                                  name=f"whhl{k}")
                    nc.sync.dma_start(out=wl, in_=whhl_d.ap()[k])
                    whh_sb.append((wh, wl))
                wpr_sb = []
                for k in range(KH):
                    w = p2w.tile([128, 2], f32, tag=f"wpr{k}", name=f"wpr{k}")
                    nc.sync.dma_start(out=w, in_=wpr_d.ap()[k])
                    wpr_sb.append(w)
                ident = p2w.tile([128, 128], f32, tag="ident")
                make_identity(nc, ident)

                hTh_prev = None
                hTl_prev = None
                c_prev = None
                for t in range(T):
                    gxt = p2g.tile([2 * BL, FOURH // 2], f32, tag="gx",
                                   name=f"gx{t}")
                    nc.sync.dma_start(out=gxt, in_=gx[t])
                    c_new = p2c.tile([2 * BL, H // 2], f32, tag="c",
                                     name=f"c{t}")
                    h_new = p2h.tile([2 * BL, H // 2], f32, tag="h",
                                     name=f"h{t}")
                    zt = {}
                    for g in range(4):
                        gs = slice(512 * g, 512 * g + 512)
                        z = p2k.tile([2 * BL, 512], f32, tag=f"z{g}",
                                     name=f"z{t}_{g}")
                        if t > 0:
                            # both h-halves of gate g computed concurrently:
                            # same stationary h^T tile at array col 0 and 64,
                            # streaming the two W chunks, outputs landing in
                            # partitions 0-63 / 64-127 of one PSUM bank.
                            pg = p2p.tile([2 * BL, 512], f32, tag=f"gp{g}",
                                          name=f"gp{t}_{g}")
                            i = 0
                            nmm = 3 * KH
                            for k in range(KH):
                                ks = slice(k * 64, (k + 1) * 64)
                                wh, wl = whh_sb[k]
                                for lhsT, rhs in ((hTh_prev, wh),
                                                  (hTl_prev, wh),
                                                  (hTh_prev, wl)):
                                    first, last = i == 0, i == nmm - 1
                                    nc.tensor.matmul(
                                        pg[0:BL, :], lhsT[:, ks],
                                        rhs[:, 1024 * g:1024 * g + 512],
                                        start=first, stop=last,
                                        tile_position=(0, 0))
                                    nc.tensor.matmul(
                                        pg[BL:2 * BL, :], lhsT[:, ks],
                                        rhs[:, 1024 * g + 512:
                                            1024 * g + 1024],
                                        start=first, stop=last,
                                        tile_position=(0, 64))
                                    i += 1
                            nc.vector.tensor_tensor(z, pg, gxt[:, gs], add)
                        else:
                            nc.vector.tensor_copy(z, gxt[:, gs])
                        nc.scalar.activation(z, z, Tanh if g == 2 else Sig)
                        zt[g] = z
                    # c = sig(f)*c_prev + sig(i)*tanh(g)  (full folded width)
                    nc.vector.tensor_tensor(zt[2], zt[0], zt[2], mult)
                    if t > 0:
                        nc.vector.tensor_tensor(zt[1], zt[1], c_prev, mult)
                        nc.vector.tensor_tensor(c_new, zt[1], zt[2], add)
                    else:
                        nc.vector.tensor_copy(c_new, zt[2])
                    nc.scalar.activation(zt[0], c_new, Tanh)
                    nc.vector.tensor_tensor(h_new, zt[3], zt[0], mult)
                    # h^T rebuild from the folded layout via regular matmuls:
                    # out = h_slice.T @ ident[:, 64hh:64hh+64] transposes and
                    # selects fold half hh in one op, all operands at
                    # partition base 0 (base-64 stationaries crash walrus).
                    # k-tile k = logical h cols [128k, 128k+128) = fold half
                    # hh=k//4, fold cols 128*(k%4)+.
                    pht = p2r.tile([128, 512], f32, tag="pht", name=f"pht{t}")
                    for k in range(KH):
                        hh, kk = divmod(k, 4)
                        nc.tensor.matmul(
                            pht[:, k * 64:(k + 1) * 64],
                            h_new[:, kk * 128:(kk + 1) * 128],
                            ident[:, 64 * hh:64 * hh + 64],
                            start=True, stop=True)
                    hT_new = p2t.tile([128, 512], f32, tag="hT",
                                      name=f"hT{t}")
                    nc.vector.tensor_copy(hT_new, pht)
                    hTh = p2t.tile([128, 512], f16, tag="hTh",
                                   name=f"hTh{t}")
                    nc.vector.tensor_copy(hTh, hT_new)
                    hTl = p2t.tile([128, 512], f16, tag="hTl",
                                   name=f"hTl{t}")
                    nc.vector.tensor_tensor(hTl, hT_new, hTh, sub)
                    pzv = p2q.tile([2, 64], f32, tag="pzv", name=f"pzv{t}")
                    for k in range(KH):
                        nc.tensor.matmul(pzv, wpr_sb[k],
                                         hT_new[:, k * 64:(k + 1) * 64],
                                         start=(k == 0), stop=(k == KH - 1))
                    zz = p2t.tile([2, 64], f32, tag="zz", name=f"zz{t}")
                    nc.vector.tensor_copy(zz, pzv)
                    nc.sync.dma_start(out=z_d.ap()[:, t * 64:(t + 1) * 64],
                                      in_=zz)
                    hTh_prev, hTl_prev, c_prev = hTh, hTl, c_new

    nc.compile()
    return nc


def _get_nc(T):
    if T not in _compiled:
        _compiled[T] = _build(T)
    return _compiled[T]


def _split16(w):
    """Split fp32 matrix into fp16 hi/lo pair with hi+lo ~= w (21 bits)."""
    wh = w.astype(np.float16)
    wl = (w - wh.astype(np.float32)).astype(np.float16)
    return wh, wl


def _prep_inputs(x, W_ih, W_hh, b_ih, b_hh, W_out, W_ctrl):
    T = x.shape[0]
    wih = np.ascontiguousarray(W_ih.T).reshape(KD, 128, FOURH)
    whh = np.ascontiguousarray(W_hh.T).reshape(KH, 128, FOURH)
    wihh, wihl = _split16(wih)
    whhh, whhl = _split16(whh)
    wpr = np.ascontiguousarray(
        np.stack([W_out[0], W_ctrl[0, :H]], axis=1)).reshape(KH, 128, 2)
    bia = np.ascontiguousarray(b_ih + b_hh)
    in_maps = []
    for r in range(NCORES):
        xt = np.ascontiguousarray(
            x[:, r * BL:(r + 1) * BL, :].transpose(2, 0, 1).reshape(D, T * BL))
        in_maps.append({"xt": xt, "wihh": wihh, "wihl": wihl,
                        "whhh": whhh, "whhl": whhl, "wpr": wpr, "bia": bia})
    return in_maps


def run_device(x, W_ih, W_hh, b_ih, b_hh, W_out, W_ctrl, trace=False):
    """Run the device part; returns Z (2, T, B) fp32 [h@w_out ; h@w_ctrl_h],
    plus the BassKernelResults (for profiling)."""
    from concourse.bass_utils import run_bass_kernel_spmd

    T = x.shape[0]
    nc = _get_nc(T)
    in_maps = _prep_inputs(x, W_ih, W_hh, b_ih, b_hh, W_out, W_ctrl)
    res = run_bass_kernel_spmd(nc, in_maps, list(range(NCORES)), trace=trace)
    Z = np.empty((2, T, B), np.float32)
    for r in range(NCORES):
        Z[:, :, r * BL:(r + 1) * BL] = res.results[r]["z"].reshape(2, T, BL)
    return Z, res


def _postprocess(Z, u, eps, b_out, W_ctrl, b_ctrl):
    T = Z.shape[1]
    e = np.float64(np.float32(eps[0]))
    logits_all = Z[0] + np.float32(b_out[0])            # (T, B) fp32
    wt = np.float64(W_ctrl[0, H])
    bc = np.float64(b_ctrl[0])
    ts_col = np.arange(T, dtype=np.float64)[:, None]
    zc = Z[1].astype(np.float64) + ts_col * wt + bc
    p = 1.0 / (1.0 + np.exp(-zc))
    p = (1.0 - e) * p + e * 0.05
    p = np.where(np.isclose(p, 0.0), p + 1e-6, p)
    a = u.astype(np.float64) < p                        # (T, B) bool
    Bn = Z.shape[2]
    preds = np.zeros(Bn, np.float64)
    hp = np.full(Bn, -1.0, np.float64)
    for t in range(T):
        halt = a[t]
        upd = halt & (preds == 0.0)
        preds = np.where(upd, logits_all[t].astype(np.float64), preds)
        hpu = (hp == -1.0) & halt
        hp = np.where(hpu, float(t), hp)
    final_logits = logits_all[T - 1].astype(np.float64)
    logits_out = np.where(preds == 0.0, final_logits, preds).astype(np.float32)
    hp2 = np.where(hp == -1.0, float(T - 1), hp)
    halting_points = (hp2 + 1.0).astype(np.float32)
    hmean = np.float32(np.mean(1.0 + hp2) / np.float64(T + 1))
    return logits_out, halting_points, hmean


def kernel(x, u, eps, W_ih, W_hh, b_ih, b_hh, W_out, b_out, W_ctrl, b_ctrl,
           W_base, b_base):
    x = np.asarray(x, np.float32)
    u = np.asarray(u, np.float32)
    Z, _ = run_device(x, np.asarray(W_ih, np.float32),
                      np.asarray(W_hh, np.float32),
                      np.asarray(b_ih, np.float32),
                      np.asarray(b_hh, np.float32),
                      np.asarray(W_out, np.float32),
                      np.asarray(W_ctrl, np.float32))
    return _postprocess(Z, u, np.asarray(eps), np.asarray(b_out),
                        np.asarray(W_ctrl), np.asarray(b_ctrl))


# revision 11
# speedup vs baseline: 2.5311x; 1.0414x over previous
"""Trainium2 Bass kernel for the EARLIEST adaptive-halting LSTM.

Shapes (hardcoded from the problem spec):
  x: (T=128, B=512, D=768), u: (T, B), eps: (1,)
  LSTM: H=1024, gates 4H=4096 (torch order i,f,g,o), C=1 output class.

Strategy:
  - Data-parallel over batch: 8 NeuronCores x 64 batch rows each.
  - The halting controller is observational (a/preds/hp never feed back into
    the recurrence), so the device only computes the LSTM scan plus the two
    per-step matvecs h@w_out and h@w_ctrl.  All epsilon-mixing / Bernoulli /
    first-halt logic runs on the host from the tiny (2, T*64) device output.
  - fp32 matmuls run at 4 cycles/row (2 half-speed HW passes, each with its
    own LDWEIGHTS).  Instead, weights and activations are split into fp16
    hi/lo pairs (W = Wh + Wl, h = hh + hl, each fp16 = 10+ mantissa bits, so
    the pair carries ~21 bits) and each matmul is computed as three
    full-rate fp16 passes Wh*hh + Wh*hl + Wl*hh accumulated in fp32 PSUM.
    That is 3 cycles/row of fp16 work vs 4 for native fp32, with ~fp32
    accuracy (validated against the reference: halting decisions exact).
  - Both phases keep the *small* operand stationary and stream the 4H-wide
    weight matrix as the moving operand (N=512 chunks, stream-bound):
      phase 1: stationary X^T blocks (M=128 of T*B), moving W_ih^T -> Gx in
               (t*b, 4H) layout at full rate.
      phase 2: stationary h^T tiles (M=64 batch), moving W_hh^T.  Gates come
               out as (B, 4H); a per-step PE-transpose pass rebuilds h^T.
"""

import sys

sys.path.insert(0, "/opt/trn_rl_repo")

import numpy as np

T_FULL, B, D, H = 128, 512, 768, 1024
NCORES = 8
BL = B // NCORES            # 64 batch rows per core
KD = D // 128               # 6 K-tiles over D
KH = H // 128               # 8 K-tiles over H
FOURH = 4 * H

_compiled = {}


def _build(T):
    import concourse.bass as bass
    import concourse.tile as tile
    from concourse import bacc, mybir
    from concourse.masks import make_identity

    f32 = mybir.dt.float32
    f16 = mybir.dt.float16
    NTB = (T * BL) // 128   # number of 128-row blocks of the T*B axis
    assert (T * BL) % 128 == 0

    nc = bacc.Bacc("TRN2", target_bir_lowering=False, debug=False,
                   num_devices=NCORES)
    xt_d = nc.dram_tensor("xt", [D, T * BL], f32, kind="ExternalInput")
    wihh_d = nc.dram_tensor("wihh", [KD, 128, FOURH], f16,
                            kind="ExternalInput")
    wihl_d = nc.dram_tensor("wihl", [KD, 128, FOURH], f16,
                            kind="ExternalInput")
    whhh_d = nc.dram_tensor("whhh", [KH, 128, FOURH], f16,
                            kind="ExternalInput")
    whhl_d = nc.dram_tensor("whhl", [KH, 128, FOURH], f16,
                            kind="ExternalInput")
    wpr_d = nc.dram_tensor("wpr", [KH, 128, 2], f32, kind="ExternalInput")
    bia_d = nc.dram_tensor("bia", [FOURH], f32, kind="ExternalInput")
    z_d = nc.dram_tensor("z", [2, T * BL], f32, kind="ExternalOutput")

    add = mybir.AluOpType.add
    sub = mybir.AluOpType.subtract
    mult = mybir.AluOpType.mult
    Sig = mybir.ActivationFunctionType.Sigmoid
    Tanh = mybir.ActivationFunctionType.Tanh

    with tile.TileContext(nc) as tc:
        with tc.tile_pool(name="dram", bufs=1, space="DRAM") as dp:
            # "Folded" gates layout: partition = batch + 64*h_half, free =
            # gate*512 + col.  Lets two M=64 matmuls run concurrently in the
            # PE array via column tiling (tile_position col 0 / 64).
            gx = dp.tile([T, 2 * BL, FOURH // 2], f32)

            # ---- phase 1: Gx[t,b,:] = x_t @ W_ih.T + (b_ih + b_hh) ----
            with tc.tile_pool(name="p1w", bufs=1) as p1w, \
                 tc.tile_pool(name="p1x", bufs=18) as p1x, \
                 tc.tile_pool(name="p1s", bufs=6) as p1s, \
                 tc.tile_pool(name="p1p", bufs=8, space="PSUM") as p1p:
                wih_sb = []
                for k in range(KD):
                    wh = p1w.tile([128, FOURH], f16, tag=f"wihh{k}",
                                  name=f"wihh{k}")
                    nc.sync.dma_start(out=wh, in_=wihh_d.ap()[k])
                    wl = p1w.tile([128, FOURH], f16, tag=f"wihl{k}",
                                  name=f"wihl{k}")
                    nc.sync.dma_start(out=wl, in_=wihl_d.ap()[k])
                    wih_sb.append((wh, wl))
                biasb = p1w.tile([128, FOURH], f32, tag="biasb")
                bsrc = bass.AP(tensor=bia_d.ap().tensor, offset=0,
                               ap=[[0, 128], [1, FOURH]])
                nc.sync.dma_start(out=biasb, in_=bsrc)
                for tbi in range(NTB):
                    xst = []
                    for k in range(KD):
                        xx = p1x.tile([128, 128], f32, tag="xst",
                                      name=f"xst{tbi}_{k}")
                        nc.sync.dma_start(
                            out=xx,
                            in_=xt_d.ap()[k * 128:(k + 1) * 128,
                                          tbi * 128:(tbi + 1) * 128])
                        xh = p1x.tile([128, 128], f16, tag="xsth",
                                      name=f"xsth{tbi}_{k}")
                        nc.vector.tensor_copy(xh, xx)
                        xl = p1x.tile([128, 128], f16, tag="xstl",
                                      name=f"xstl{tbi}_{k}")
                        nc.vector.tensor_tensor(xl, xx, xh, sub)
                        xst.append((xh, xl))
                    for c8 in range(8):
                        cs = slice(c8 * 512, (c8 + 1) * 512)
                        ps = p1p.tile([128, 512], f32, tag="ps",
                                      name=f"ps{tbi}_{c8}")
                        nmm = 3 * KD
                        i = 0
                        for k in range(KD):
                            xh, xl = xst[k]
                            wh, wl = wih_sb[k]
                            for lhsT, rhs in ((xh, wh), (xl, wh), (xh, wl)):
                                nc.tensor.matmul(ps, lhsT, rhs[:, cs],
                                                 start=(i == 0),
                                                 stop=(i == nmm - 1))
                                i += 1
                        st = p1s.tile([128, 512], f32, tag="st",
                                      name=f"st{tbi}_{c8}")
                        nc.vector.tensor_tensor(st, ps, biasb[:, cs], add)
                        g8, hh8 = divmod(c8, 2)
                        dst = gx[2 * tbi:2 * tbi + 2,
                                 64 * hh8:64 * hh8 + 64,
                                 512 * g8:512 * g8 + 512]
                        nc.sync.dma_start(out=dst, in_=st)

            # ---- phase 2: LSTM scan + [w_out, w_ctrl] matvec per step ----
            with tc.tile_pool(name="p2w", bufs=1) as p2w, \
                 tc.tile_pool(name="p2g", bufs=2) as p2g, \
                 tc.tile_pool(name="p2h", bufs=2) as p2h, \
                 tc.tile_pool(name="p2c", bufs=2) as p2c, \
                 tc.tile_pool(name="p2k", bufs=2) as p2k, \
                 tc.tile_pool(name="p2t", bufs=2) as p2t, \
                 tc.tile_pool(name="p2p", bufs=1, space="PSUM") as p2p, \
                 tc.tile_pool(name="p2r", bufs=2, space="PSUM") as p2r, \
                 tc.tile_pool(name="p2q", bufs=2, space="PSUM") as p2q:
                whh_sb = []
                for k in range(KH):
                    wh = p2w.tile([128, FOURH], f16, tag=f"whhh{k}",
                                  name=f"whhh{k}")
                    nc.sync.dma_start(out=wh, in_=whhh_d.ap()[k])
                    wl = p2w.tile([128, FOURH], f16, tag=f"whhl{k}",
                                  name=f"whhl{k}")
                    nc.sync.dma_start(out=wl, in_=whhl_d.ap()[k])
                    whh_sb.append((wh, wl))
                wpr_sb = []
                for k in range(KH):
                    w = p2w.tile([128, 2], f32, tag=f"wpr{k}", name=f"wpr{k}")
                    nc.sync.dma_start(out=w, in_=wpr_d.ap()[k])
                    wpr_sb.append(w)
                ident = p2w.tile([128, 128], f32, tag="ident")
                make_identity(nc, ident)

                hTh_prev = None
                hTl_prev = None
                c_prev = None
                for t in range(T):
                    gxt = p2g.tile([2 * BL, FOURH // 2], f32, tag="gx",
                                   name=f"gx{t}")
                    nc.sync.dma_start(out=gxt, in_=gx[t])
                    c_new = p2c.tile([2 * BL, H // 2], f32, tag="c",
                                     name=f"c{t}")
                    h_new = p2h.tile([2 * BL, H // 2], f32, tag="h",
                                     name=f"h{t}")
                    zt = {}
                    for g in range(4):
                        gs = slice(512 * g, 512 * g + 512)
                        z = p2k.tile([2 * BL, 512], f32, tag=f"z{g}",
                                     name=f"z{t}_{g}")
                        if t > 0:
                            # both h-halves of gate g computed concurrently:
                            # same stationary h^T tile at array col 0 and 64,
                            # streaming the two W chunks, outputs landing in
                            # partitions 0-63 / 64-127 of one PSUM bank.
                            pg = p2p.tile([2 * BL, 512], f32, tag=f"gp{g}",
                                          name=f"gp{t}_{g}")
                            i = 0
                            nmm = 3 * KH
                            for k in range(KH):
                                ks = slice(k * 64, (k + 1) * 64)
                                wh, wl = whh_sb[k]
                                for lhsT, rhs in ((hTh_prev, wh),
                                                  (hTl_prev, wh),
                                                  (hTh_prev, wl)):
                                    first, last = i == 0, i == nmm - 1
                                    nc.tensor.matmul(
                                        pg[0:BL, :], lhsT[:, ks],
                                        rhs[:, 1024 * g:1024 * g + 512],
                                        start=first, stop=last,
                                        tile_position=(0, 0))
                                    nc.tensor.matmul(
                                        pg[BL:2 * BL, :], lhsT[:, ks],
                                        rhs[:, 1024 * g + 512:
                                            1024 * g + 1024],
                                        start=first, stop=last,
                                        tile_position=(0, 64))
                                    i += 1
                            nc.vector.tensor_tensor(z, pg, gxt[:, gs], add)
                        else:
                            nc.vector.tensor_copy(z, gxt[:, gs])
                        nc.scalar.activation(z, z, Tanh if g == 2 else Sig)
                        zt[g] = z
                    # c = sig(f)*c_prev + sig(i)*tanh(g)  (full folded width)
                    nc.vector.tensor_tensor(zt[2], zt[0], zt[2], mult)
                    if t > 0:
                        nc.vector.tensor_tensor(zt[1], zt[1], c_prev, mult)
                        nc.vector.tensor_tensor(c_new, zt[1], zt[2], add)
                    else:
                        nc.vector.tensor_copy(c_new, zt[2])
                    nc.scalar.activation(zt[0], c_new, Tanh)
                    nc.vector.tensor_tensor(h_new, zt[3], zt[0], mult)
                    # h^T rebuild from the folded layout via regular matmuls:
                    # out = h_slice.T @ ident[:, 64hh:64hh+64] transposes and
                    # selects fold half hh in one op, all operands at
                    # partition base 0 (base-64 stationaries crash walrus).
                    # k-tile k = logical h cols [128k, 128k+128) = fold half
                    # hh=k//4, fold cols 128*(k%4)+.
                    pht = p2r.tile([128, 512], f32, tag="pht", name=f"pht{t}")
                    for k in range(KH):
                        hh, kk = divmod(k, 4)
                        nc.tensor.matmul(
                            pht[:, k * 64:(k + 1) * 64],
                            h_new[:, kk * 128:(kk + 1) * 128],
                            ident[:, 64 * hh:64 * hh + 64],
                            start=True, stop=True)
                    # fp16 hi/lo casts straight from PSUM so the next step's
                    # matmuls aren't gated on the fp32 copy (matvec-only).
                    hTh = p2t.tile([128, 512], f16, tag="hTh",
                                   name=f"hTh{t}")
                    nc.vector.tensor_copy(hTh, pht)
                    hTl = p2t.tile([128, 512], f16, tag="hTl",
                                   name=f"hTl{t}")
                    nc.vector.tensor_tensor(hTl, pht, hTh, sub)
                    hT_new = p2t.tile([128, 512], f32, tag="hT",
                                      name=f"hT{t}")
                    nc.vector.tensor_copy(hT_new, pht)
                    pzv = p2q.tile([2, 64], f32, tag="pzv", name=f"pzv{t}")
                    for k in range(KH):
                        nc.tensor.matmul(pzv, wpr_sb[k],
                                         hT_new[:, k * 64:(k + 1) * 64],
                                         start=(k == 0), stop=(k == KH - 1))
                    zz = p2t.tile([2, 64], f32, tag="zz", name=f"zz{t}")
                    nc.vector.tensor_copy(zz, pzv)
                    nc.sync.dma_start(out=z_d.ap()[:, t * 64:(t + 1) * 64],
                                      in_=zz)
                    hTh_prev, hTl_prev, c_prev = hTh, hTl, c_new

    nc.compile()
    return nc


def _get_nc(T):
    if T not in _compiled:
        _compiled[T] = _build(T)
    return _compiled[T]


def _split16(w):
    """Split fp32 matrix into fp16 hi/lo pair with hi+lo ~= w (21 bits)."""
    wh = w.astype(np.float16)
    wl = (w - wh.astype(np.float32)).astype(np.float16)
    return wh, wl


def _prep_inputs(x, W_ih, W_hh, b_ih, b_hh, W_out, W_ctrl):
    T = x.shape[0]
    wih = np.ascontiguousarray(W_ih.T).reshape(KD, 128, FOURH)
    whh = np.ascontiguousarray(W_hh.T).reshape(KH, 128, FOURH)
    wihh, wihl = _split16(wih)
    whhh, whhl = _split16(whh)
    wpr = np.ascontiguousarray(
        np.stack([W_out[0], W_ctrl[0, :H]], axis=1)).reshape(KH, 128, 2)
    bia = np.ascontiguousarray(b_ih + b_hh)
    in_maps = []
    for r in range(NCORES):
        xt = np.ascontiguousarray(
            x[:, r * BL:(r + 1) * BL, :].transpose(2, 0, 1).reshape(D, T * BL))
        in_maps.append({"xt": xt, "wihh": wihh, "wihl": wihl,
                        "whhh": whhh, "whhl": whhl, "wpr": wpr, "bia": bia})
    return in_maps


def run_device(x, W_ih, W_hh, b_ih, b_hh, W_out, W_ctrl, trace=False):
    """Run the device part; returns Z (2, T, B) fp32 [h@w_out ; h@w_ctrl_h],
    plus the BassKernelResults (for profiling)."""
    from concourse.bass_utils import run_bass_kernel_spmd

    T = x.shape[0]
    nc = _get_nc(T)
    in_maps = _prep_inputs(x, W_ih, W_hh, b_ih, b_hh, W_out, W_ctrl)
    res = run_bass_kernel_spmd(nc, in_maps, list(range(NCORES)), trace=trace)
    Z = np.empty((2, T, B), np.float32)
    for r in range(NCORES):
        Z[:, :, r * BL:(r + 1) * BL] = res.results[r]["z"].reshape(2, T, BL)
    return Z, res


def _postprocess(Z, u, eps, b_out, W_ctrl, b_ctrl):
    T = Z.shape[1]
    e = np.float64(np.float32(eps[0]))
    logits_all = Z[0] + np.float32(b_out[0])            # (T, B) fp32
    wt = np.float64(W_ctrl[0, H])
    bc = np.float64(b_ctrl[0])
    ts_col = np.arange(T, dtype=np.float64)[:, None]
    zc = Z[1].astype(np.float64) + ts_col * wt + bc
    p = 1.0 / (1.0 + np.exp(-zc))
    p = (1.0 - e) * p + e * 0.05
    p = np.where(np.isclose(p, 0.0), p + 1e-6, p)
    a = u.astype(np.float64) < p                        # (T, B) bool
    Bn = Z.shape[2]
    preds = np.zeros(Bn, np.float64)
    hp = np.full(Bn, -1.0, np.float64)
    for t in range(T):
        halt = a[t]
        upd = halt & (preds == 0.0)
        preds = np.where(upd, logits_all[t].astype(np.float64), preds)
        hpu = (hp == -1.0) & halt
        hp = np.where(hpu, float(t), hp)
    final_logits = logits_all[T - 1].astype(np.float64)
    logits_out = np.where(preds == 0.0, final_logits, preds).astype(np.float32)
    hp2 = np.where(hp == -1.0, float(T - 1), hp)
    halting_points = (hp2 + 1.0).astype(np.float32)
    hmean = np.float32(np.mean(1.0 + hp2) / np.float64(T + 1))
    return logits_out, halting_points, hmean


def kernel(x, u, eps, W_ih, W_hh, b_ih, b_hh, W_out, b_out, W_ctrl, b_ctrl,
           W_base, b_base):
    x = np.asarray(x, np.float32)
    u = np.asarray(u, np.float32)
    Z, _ = run_device(x, np.asarray(W_ih, np.float32),
                      np.asarray(W_hh, np.float32),
                      np.asarray(b_ih, np.float32),
                      np.asarray(b_hh, np.float32),
                      np.asarray(W_out, np.float32),
                      np.asarray(W_ctrl, np.float32))
    return _postprocess(Z, u, np.asarray(eps), np.asarray(b_out),
                        np.asarray(W_ctrl), np.asarray(b_ctrl))


# revision 14
# speedup vs baseline: 2.6340x; 1.0407x over previous
"""Trainium2 Bass kernel for the EARLIEST adaptive-halting LSTM.

Shapes (hardcoded from the problem spec):
  x: (T=128, B=512, D=768), u: (T, B), eps: (1,)
  LSTM: H=1024, gates 4H=4096 (torch order i,f,g,o), C=1 output class.

Strategy:
  - Data-parallel over batch: 8 NeuronCores x 64 batch rows each.
  - The halting controller is observational (a/preds/hp never feed back into
    the recurrence), so the device only computes the LSTM scan plus the two
    per-step matvecs h@w_out and h@w_ctrl.  All epsilon-mixing / Bernoulli /
    first-halt logic runs on the host from the tiny (2, T*64) device output.
  - fp32 matmuls run at 4 cycles/row (2 half-speed HW passes, each with its
    own LDWEIGHTS).  Instead, weights and activations are split into fp16
    hi/lo pairs (W = Wh + Wl, h = hh + hl, each fp16 = 10+ mantissa bits, so
    the pair carries ~21 bits) and each matmul is computed as three
    full-rate fp16 passes Wh*hh + Wh*hl + Wl*hh accumulated in fp32 PSUM.
    That is 3 cycles/row of fp16 work vs 4 for native fp32, with ~fp32
    accuracy (validated against the reference: halting decisions exact).
  - Both phases keep the *small* operand stationary and stream the 4H-wide
    weight matrix as the moving operand (N=512 chunks, stream-bound):
      phase 1: stationary X^T blocks (M=128 of T*B), moving W_ih^T -> Gx in
               (t*b, 4H) layout at full rate.
      phase 2: stationary h^T tiles (M=64 batch), moving W_hh^T.  Gates come
               out as (B, 4H); a per-step PE-transpose pass rebuilds h^T.
"""

import sys

sys.path.insert(0, "/opt/trn_rl_repo")

import numpy as np

T_FULL, B, D, H = 128, 512, 768, 1024
NCORES = 8
BL = B // NCORES            # 64 batch rows per core
KD = D // 128               # 6 K-tiles over D
KH = H // 128               # 8 K-tiles over H
FOURH = 4 * H

_compiled = {}


def _build(T):
    import concourse.bass as bass
    import concourse.tile as tile
    from concourse import bacc, mybir
    from concourse.masks import make_identity

    f32 = mybir.dt.float32
    f16 = mybir.dt.float16
    NTB = (T * BL) // 128   # number of 128-row blocks of the T*B axis
    assert (T * BL) % 128 == 0

    nc = bacc.Bacc("TRN2", target_bir_lowering=False, debug=False,
                   num_devices=NCORES)
    xt_d = nc.dram_tensor("xt", [D, T * BL], f32, kind="ExternalInput")
    wihh_d = nc.dram_tensor("wihh", [KD, 128, FOURH], f16,
                            kind="ExternalInput")
    wihl_d = nc.dram_tensor("wihl", [KD, 128, FOURH], f16,
                            kind="ExternalInput")
    whhh_d = nc.dram_tensor("whhh", [KH, 128, FOURH], f16,
                            kind="ExternalInput")
    whhl_d = nc.dram_tensor("whhl", [KH, 128, FOURH], f16,
                            kind="ExternalInput")
    wpr_d = nc.dram_tensor("wpr", [KH, 128, 2], f32, kind="ExternalInput")
    bia_d = nc.dram_tensor("bia", [FOURH], f32, kind="ExternalInput")
    z_d = nc.dram_tensor("z", [2, T * BL], f32, kind="ExternalOutput")

    add = mybir.AluOpType.add
    sub = mybir.AluOpType.subtract
    mult = mybir.AluOpType.mult
    Sig = mybir.ActivationFunctionType.Sigmoid
    Tanh = mybir.ActivationFunctionType.Tanh

    with tile.TileContext(nc) as tc:
        with tc.tile_pool(name="dram", bufs=1, space="DRAM") as dp:
            # "Folded" gates layout: partition = batch + 64*h_half, free =
            # gate*512 + col.  Lets two M=64 matmuls run concurrently in the
            # PE array via column tiling (tile_position col 0 / 64).
            gx = dp.tile([T, 2 * BL, FOURH // 2], f32)

            # ---- phase 1: Gx[t,b,:] = x_t @ W_ih.T + (b_ih + b_hh) ----
            with tc.tile_pool(name="p1w", bufs=1) as p1w, \
                 tc.tile_pool(name="p1x", bufs=18) as p1x, \
                 tc.tile_pool(name="p1s", bufs=6) as p1s, \
                 tc.tile_pool(name="p1p", bufs=8, space="PSUM") as p1p:
                wih_sb = []
                for k in range(KD):
                    wh = p1w.tile([128, FOURH], f16, tag=f"wihh{k}",
                                  name=f"wihh{k}")
                    nc.sync.dma_start(out=wh, in_=wihh_d.ap()[k])
                    wl = p1w.tile([128, FOURH], f16, tag=f"wihl{k}",
                                  name=f"wihl{k}")
                    nc.sync.dma_start(out=wl, in_=wihl_d.ap()[k])
                    wih_sb.append((wh, wl))
                biasb = p1w.tile([128, FOURH], f32, tag="biasb")
                bsrc = bass.AP(tensor=bia_d.ap().tensor, offset=0,
                               ap=[[0, 128], [1, FOURH]])
                nc.sync.dma_start(out=biasb, in_=bsrc)
                for tbi in range(NTB):
                    xst = []
                    for k in range(KD):
                        xx = p1x.tile([128, 128], f32, tag="xst",
                                      name=f"xst{tbi}_{k}")
                        nc.sync.dma_start(
                            out=xx,
                            in_=xt_d.ap()[k * 128:(k + 1) * 128,
                                          tbi * 128:(tbi + 1) * 128])
                        xh = p1x.tile([128, 128], f16, tag="xsth",
                                      name=f"xsth{tbi}_{k}")
                        nc.vector.tensor_copy(xh, xx)
                        xl = p1x.tile([128, 128], f16, tag="xstl",
                                      name=f"xstl{tbi}_{k}")
                        nc.vector.tensor_tensor(xl, xx, xh, sub)
                        xst.append((xh, xl))
                    for c8 in range(8):
                        cs = slice(c8 * 512, (c8 + 1) * 512)
                        ps = p1p.tile([128, 512], f32, tag="ps",
                                      name=f"ps{tbi}_{c8}")
                        nmm = 3 * KD
                        i = 0
                        for k in range(KD):
                            xh, xl = xst[k]
                            wh, wl = wih_sb[k]
                            for lhsT, rhs in ((xh, wh), (xl, wh), (xh, wl)):
                                nc.tensor.matmul(ps, lhsT, rhs[:, cs],
                                                 start=(i == 0),
                                                 stop=(i == nmm - 1))
                                i += 1
                        st = p1s.tile([128, 512], f32, tag="st",
                                      name=f"st{tbi}_{c8}")
                        nc.vector.tensor_tensor(st, ps, biasb[:, cs], add)
                        g8, hh8 = divmod(c8, 2)
                        dst = gx[2 * tbi:2 * tbi + 2,
                                 64 * hh8:64 * hh8 + 64,
                                 512 * g8:512 * g8 + 512]
                        nc.sync.dma_start(out=dst, in_=st)

            # ---- phase 2: LSTM scan + [w_out, w_ctrl] matvec per step ----
            with tc.tile_pool(name="p2w", bufs=1) as p2w, \
                 tc.tile_pool(name="p2g", bufs=2) as p2g, \
                 tc.tile_pool(name="p2h", bufs=2) as p2h, \
                 tc.tile_pool(name="p2c", bufs=2) as p2c, \
                 tc.tile_pool(name="p2k", bufs=2) as p2k, \
                 tc.tile_pool(name="p2t", bufs=2) as p2t, \
                 tc.tile_pool(name="p2p", bufs=1, space="PSUM") as p2p, \
                 tc.tile_pool(name="p2r", bufs=2, space="PSUM") as p2r, \
                 tc.tile_pool(name="p2q", bufs=2, space="PSUM") as p2q:
                whh_sb = []
                for k in range(KH):
                    wh = p2w.tile([128, FOURH], f16, tag=f"whhh{k}",
                                  name=f"whhh{k}")
                    nc.sync.dma_start(out=wh, in_=whhh_d.ap()[k])
                    wl = p2w.tile([128, FOURH], f16, tag=f"whhl{k}",
                                  name=f"whhl{k}")
                    nc.sync.dma_start(out=wl, in_=whhl_d.ap()[k])
                    whh_sb.append((wh, wl))
                wpr_sb = []
                for k in range(KH):
                    w = p2w.tile([128, 2], f32, tag=f"wpr{k}", name=f"wpr{k}")
                    nc.sync.dma_start(out=w, in_=wpr_d.ap()[k])
                    wpr_sb.append(w)
                ident = p2w.tile([128, 128], f32, tag="ident")
                make_identity(nc, ident)

                def emit_matvec(tv, hT):
                    # z_t = [w_out, w_ctrl] @ h_t ; deferred one step so the
                    # PE runs it behind the next step's main matmuls instead
                    # of on the recurrence critical path.
                    pzv = p2q.tile([2, 64], f32, tag="pzv", name=f"pzv{tv}")
                    for k in range(KH):
                        nc.tensor.matmul(pzv, wpr_sb[k],
                                         hT[:, k * 64:(k + 1) * 64],
                                         start=(k == 0), stop=(k == KH - 1))
                    zz = p2t.tile([2, 64], f32, tag="zz", name=f"zz{tv}")
                    nc.vector.tensor_copy(zz, pzv)
                    nc.sync.dma_start(
                        out=z_d.ap()[:, tv * 64:(tv + 1) * 64], in_=zz)

                hTh_prev = None
                hTl_prev = None
                c_prev = None
                pending = None
                for t in range(T):
                    gxt = p2g.tile([2 * BL, FOURH // 2], f32, tag="gx",
                                   name=f"gx{t}")
                    nc.sync.dma_start(out=gxt, in_=gx[t])
                    c_new = p2c.tile([2 * BL, H // 2], f32, tag="c",
                                     name=f"c{t}")
                    h_new = p2h.tile([2 * BL, H // 2], f32, tag="h",
                                     name=f"h{t}")
                    zt = {}
                    for g in range(4):
                        gs = slice(512 * g, 512 * g + 512)
                        z = p2k.tile([2 * BL, 512], f32, tag=f"z{g}",
                                     name=f"z{t}_{g}")
                        if t > 0:
                            # both h-halves of gate g computed concurrently:
                            # same stationary h^T tile at array col 0 and 64,
                            # streaming the two W chunks, outputs landing in
                            # partitions 0-63 / 64-127 of one PSUM bank.
                            pg = p2p.tile([2 * BL, 512], f32, tag=f"gp{g}",
                                          name=f"gp{t}_{g}")
                            i = 0
                            nmm = 3 * KH
                            for k in range(KH):
                                ks = slice(k * 64, (k + 1) * 64)
                                wh, wl = whh_sb[k]
                                for lhsT, rhs in ((hTh_prev, wh),
                                                  (hTl_prev, wh),
                                                  (hTh_prev, wl)):
                                    first, last = i == 0, i == nmm - 1
                                    nc.tensor.matmul(
                                        pg[0:BL, :], lhsT[:, ks],
                                        rhs[:, 1024 * g:1024 * g + 512],
                                        start=first, stop=last,
                                        tile_position=(0, 0))
                                    nc.tensor.matmul(
                                        pg[BL:2 * BL, :], lhsT[:, ks],
                                        rhs[:, 1024 * g + 512:
                                            1024 * g + 1024],
                                        start=first, stop=last,
                                        tile_position=(0, 64))
                                    i += 1
                            nc.vector.tensor_tensor(z, pg, gxt[:, gs], add)
                        else:
                            nc.vector.tensor_copy(z, gxt[:, gs])
                        nc.scalar.activation(z, z, Tanh if g == 2 else Sig)
                        zt[g] = z
                    if pending is not None:
                        emit_matvec(*pending)
                        pending = None
                    # c = sig(f)*c_prev + sig(i)*tanh(g)  (full folded width)
                    nc.vector.tensor_tensor(zt[2], zt[0], zt[2], mult)
                    if t > 0:
                        nc.vector.tensor_tensor(zt[1], zt[1], c_prev, mult)
                        nc.vector.tensor_tensor(c_new, zt[1], zt[2], add)
                    else:
                        nc.vector.tensor_copy(c_new, zt[2])
                    nc.scalar.activation(zt[0], c_new, Tanh)
                    nc.vector.tensor_tensor(h_new, zt[3], zt[0], mult)
                    # h^T rebuild from the folded layout via regular matmuls:
                    # out = h_slice.T @ ident[:, 64hh:64hh+64] transposes and
                    # selects fold half hh in one op, all operands at
                    # partition base 0 (base-64 stationaries crash walrus).
                    # k-tile k = logical h cols [128k, 128k+128) = fold half
                    # hh=k//4, fold cols 128*(k%4)+.
                    pht = p2r.tile([128, 512], f32, tag="pht", name=f"pht{t}")
                    for k in range(KH):
                        hh, kk = divmod(k, 4)
                        nc.tensor.matmul(
                            pht[:, k * 64:(k + 1) * 64],
                            h_new[:, kk * 128:(kk + 1) * 128],
                            ident[:, 64 * hh:64 * hh + 64],
                            start=True, stop=True)
                    # fp16 hi/lo casts straight from PSUM so the next step's
                    # matmuls aren't gated on the fp32 copy (matvec-only).
                    hTh = p2t.tile([128, 512], f16, tag="hTh",
                                   name=f"hTh{t}")
                    nc.vector.tensor_copy(hTh, pht)
                    hTl = p2t.tile([128, 512], f16, tag="hTl",
                                   name=f"hTl{t}")
                    nc.vector.tensor_tensor(hTl, pht, hTh, sub)
                    hT_new = p2t.tile([128, 512], f32, tag="hT",
                                      name=f"hT{t}")
                    nc.vector.tensor_copy(hT_new, pht)
                    pending = (t, hT_new)
                    hTh_prev, hTl_prev, c_prev = hTh, hTl, c_new
                if pending is not None:
                    emit_matvec(*pending)

    nc.compile()
    return nc


def _get_nc(T):
    if T not in _compiled:
        _compiled[T] = _build(T)
    return _compiled[T]


def _split16(w):
    """Split fp32 matrix into fp16 hi/lo pair with hi+lo ~= w (21 bits)."""
    wh = w.astype(np.float16)
    wl = (w - wh.astype(np.float32)).astype(np.float16)
    return wh, wl


def _prep_inputs(x, W_ih, W_hh, b_ih, b_hh, W_out, W_ctrl):
    T = x.shape[0]
    wih = np.ascontiguousarray(W_ih.T).reshape(KD, 128, FOURH)
    whh = np.ascontiguousarray(W_hh.T).reshape(KH, 128, FOURH)
    wihh, wihl = _split16(wih)
    whhh, whhl = _split16(whh)
    wpr = np.ascontiguousarray(
        np.stack([W_out[0], W_ctrl[0, :H]], axis=1)).reshape(KH, 128, 2)
    bia = np.ascontiguousarray(b_ih + b_hh)
    in_maps = []
    for r in range(NCORES):
        xt = np.ascontiguousarray(
            x[:, r * BL:(r + 1) * BL, :].transpose(2, 0, 1).reshape(D, T * BL))
        in_maps.append({"xt": xt, "wihh": wihh, "wihl": wihl,
                        "whhh": whhh, "whhl": whhl, "wpr": wpr, "bia": bia})
    return in_maps


def run_device(x, W_ih, W_hh, b_ih, b_hh, W_out, W_ctrl, trace=False):
    """Run the device part; returns Z (2, T, B) fp32 [h@w_out ; h@w_ctrl_h],
    plus the BassKernelResults (for profiling)."""
    from concourse.bass_utils import run_bass_kernel_spmd

    T = x.shape[0]
    nc = _get_nc(T)
    in_maps = _prep_inputs(x, W_ih, W_hh, b_ih, b_hh, W_out, W_ctrl)
    res = run_bass_kernel_spmd(nc, in_maps, list(range(NCORES)), trace=trace)
    Z = np.empty((2, T, B), np.float32)
    for r in range(NCORES):
        Z[:, :, r * BL:(r + 1) * BL] = res.results[r]["z"].reshape(2, T, BL)
    return Z, res


def _postprocess(Z, u, eps, b_out, W_ctrl, b_ctrl):
    T = Z.shape[1]
    e = np.float64(np.float32(eps[0]))
    logits_all = Z[0] + np.float32(b_out[0])            # (T, B) fp32
    wt = np.float64(W_ctrl[0, H])
    bc = np.float64(b_ctrl[0])
    ts_col = np.arange(T, dtype=np.float64)[:, None]
    zc = Z[1].astype(np.float64) + ts_col * wt + bc
    p = 1.0 / (1.0 + np.exp(-zc))
    p = (1.0 - e) * p + e * 0.05
    p = np.where(np.isclose(p, 0.0), p + 1e-6, p)
    a = u.astype(np.float64) < p                        # (T, B) bool
    Bn = Z.shape[2]
    preds = np.zeros(Bn, np.float64)
    hp = np.full(Bn, -1.0, np.float64)
    for t in range(T):
        halt = a[t]
        upd = halt & (preds == 0.0)
        preds = np.where(upd, logits_all[t].astype(np.float64), preds)
        hpu = (hp == -1.0) & halt
        hp = np.where(hpu, float(t), hp)
    final_logits = logits_all[T - 1].astype(np.float64)
    logits_out = np.where(preds == 0.0, final_logits, preds).astype(np.float32)
    hp2 = np.where(hp == -1.0, float(T - 1), hp)
    halting_points = (hp2 + 1.0).astype(np.float32)
    hmean = np.float32(np.mean(1.0 + hp2) / np.float64(T + 1))
    return logits_out, halting_points, hmean


def kernel(x, u, eps, W_ih, W_hh, b_ih, b_hh, W_out, b_out, W_ctrl, b_ctrl,
           W_base, b_base):
    x = np.asarray(x, np.float32)
    u = np.asarray(u, np.float32)
    Z, _ = run_device(x, np.asarray(W_ih, np.float32),
                      np.asarray(W_hh, np.float32),
                      np.asarray(b_ih, np.float32),
                      np.asarray(b_hh, np.float32),
                      np.asarray(W_out, np.float32),
                      np.asarray(W_ctrl, np.float32))
    return _postprocess(Z, u, np.asarray(eps), np.asarray(b_out),
                        np.asarray(W_ctrl), np.asarray(b_ctrl))


# revision 17
# speedup vs baseline: 2.6883x; 1.0206x over previous
"""Trainium2 Bass kernel for the EARLIEST adaptive-halting LSTM.

Shapes (hardcoded from the problem spec):
  x: (T=128, B=512, D=768), u: (T, B), eps: (1,)
  LSTM: H=1024, gates 4H=4096 (torch order i,f,g,o), C=1 output class.

Strategy:
  - Data-parallel over batch: 8 NeuronCores x 64 batch rows each.
  - The halting controller is observational (a/preds/hp never feed back into
    the recurrence), so the device only computes the LSTM scan plus the two
    per-step matvecs h@w_out and h@w_ctrl.  All epsilon-mixing / Bernoulli /
    first-halt logic runs on the host from the tiny (2, T*64) device output.
  - fp32 matmuls run at 4 cycles/row (2 half-speed HW passes, each with its
    own LDWEIGHTS).  Instead, weights and activations are split into fp16
    hi/lo pairs (W = Wh + Wl, h = hh + hl, each fp16 = 10+ mantissa bits, so
    the pair carries ~21 bits) and each matmul is computed as three
    full-rate fp16 passes Wh*hh + Wh*hl + Wl*hh accumulated in fp32 PSUM.
    That is 3 cycles/row of fp16 work vs 4 for native fp32, with ~fp32
    accuracy (validated against the reference: halting decisions exact).
  - Both phases keep the *small* operand stationary and stream the 4H-wide
    weight matrix as the moving operand (N=512 chunks, stream-bound):
      phase 1: stationary X^T blocks (M=128 of T*B), moving W_ih^T -> Gx in
               (t*b, 4H) layout at full rate.
      phase 2: stationary h^T tiles (M=64 batch), moving W_hh^T.  Gates come
               out as (B, 4H); a per-step PE-transpose pass rebuilds h^T.
"""

import sys

sys.path.insert(0, "/opt/trn_rl_repo")

import numpy as np

T_FULL, B, D, H = 128, 512, 768, 1024
NCORES = 8
BL = B // NCORES            # 64 batch rows per core
KD = D // 128               # 6 K-tiles over D
KH = H // 128               # 8 K-tiles over H
FOURH = 4 * H

_compiled = {}


def _build(T):
    import concourse.bass as bass
    import concourse.tile as tile
    from concourse import bacc, mybir
    from concourse.masks import make_identity

    f32 = mybir.dt.float32
    f16 = mybir.dt.float16
    NTB = (T * BL) // 128   # number of 128-row blocks of the T*B axis
    assert (T * BL) % 128 == 0

    nc = bacc.Bacc("TRN2", target_bir_lowering=False, debug=False,
                   num_devices=NCORES)
    xt_d = nc.dram_tensor("xt", [D, T * BL], f32, kind="ExternalInput")
    wihh_d = nc.dram_tensor("wihh", [KD, 128, FOURH], f16,
                            kind="ExternalInput")
    wihl_d = nc.dram_tensor("wihl", [KD, 128, FOURH], f16,
                            kind="ExternalInput")
    whhh_d = nc.dram_tensor("whhh", [KH, 128, FOURH], f16,
                            kind="ExternalInput")
    whhl_d = nc.dram_tensor("whhl", [KH, 128, FOURH], f16,
                            kind="ExternalInput")
    wpr_d = nc.dram_tensor("wpr", [KH, 128, 2], f32, kind="ExternalInput")
    bia_d = nc.dram_tensor("bia", [FOURH], f32, kind="ExternalInput")
    z_d = nc.dram_tensor("z", [2, T * BL], f32, kind="ExternalOutput")

    add = mybir.AluOpType.add
    sub = mybir.AluOpType.subtract
    mult = mybir.AluOpType.mult
    Sig = mybir.ActivationFunctionType.Sigmoid
    Tanh = mybir.ActivationFunctionType.Tanh

    with tile.TileContext(nc) as tc:
        with tc.tile_pool(name="dram", bufs=1, space="DRAM") as dp:
            # "Folded" gates layout: partition = batch + 64*h_half, free =
            # gate*512 + col.  Lets two M=64 matmuls run concurrently in the
            # PE array via column tiling (tile_position col 0 / 64).
            gx = dp.tile([T, 2 * BL, FOURH // 2], f32)

            # ---- phase 1: Gx[t,b,:] = x_t @ W_ih.T + (b_ih + b_hh) ----
            with tc.tile_pool(name="p1w", bufs=1) as p1w, \
                 tc.tile_pool(name="p1x", bufs=18) as p1x, \
                 tc.tile_pool(name="p1s", bufs=6) as p1s, \
                 tc.tile_pool(name="p1p", bufs=8, space="PSUM") as p1p:
                wih_sb = []
                for k in range(KD):
                    wh = p1w.tile([128, FOURH], f16, tag=f"wihh{k}",
                                  name=f"wihh{k}")
                    nc.sync.dma_start(out=wh, in_=wihh_d.ap()[k])
                    wl = p1w.tile([128, FOURH], f16, tag=f"wihl{k}",
                                  name=f"wihl{k}")
                    nc.sync.dma_start(out=wl, in_=wihl_d.ap()[k])
                    wih_sb.append((wh, wl))
                biasb = p1w.tile([128, FOURH], f32, tag="biasb")
                bsrc = bass.AP(tensor=bia_d.ap().tensor, offset=0,
                               ap=[[0, 128], [1, FOURH]])
                nc.sync.dma_start(out=biasb, in_=bsrc)
                for tbi in range(NTB):
                    xst = []
                    for k in range(KD):
                        xx = p1x.tile([128, 128], f32, tag="xst",
                                      name=f"xst{tbi}_{k}")
                        nc.sync.dma_start(
                            out=xx,
                            in_=xt_d.ap()[k * 128:(k + 1) * 128,
                                          tbi * 128:(tbi + 1) * 128])
                        xh = p1x.tile([128, 128], f16, tag="xsth",
                                      name=f"xsth{tbi}_{k}")
                        nc.vector.tensor_copy(xh, xx)
                        xl = p1x.tile([128, 128], f16, tag="xstl",
                                      name=f"xstl{tbi}_{k}")
                        nc.vector.tensor_tensor(xl, xx, xh, sub)
                        xst.append((xh, xl))
                    for c8 in range(8):
                        cs = slice(c8 * 512, (c8 + 1) * 512)
                        ps = p1p.tile([128, 512], f32, tag="ps",
                                      name=f"ps{tbi}_{c8}")
                        nmm = 3 * KD
                        i = 0
                        for k in range(KD):
                            xh, xl = xst[k]
                            wh, wl = wih_sb[k]
                            for lhsT, rhs in ((xh, wh), (xl, wh), (xh, wl)):
                                nc.tensor.matmul(ps, lhsT, rhs[:, cs],
                                                 start=(i == 0),
                                                 stop=(i == nmm - 1))
                                i += 1
                        st = p1s.tile([128, 512], f32, tag="st",
                                      name=f"st{tbi}_{c8}")
                        nc.vector.tensor_tensor(st, ps, biasb[:, cs], add)
                        g8, hh8 = divmod(c8, 2)
                        dst = gx[2 * tbi:2 * tbi + 2,
                                 64 * hh8:64 * hh8 + 64,
                                 512 * g8:512 * g8 + 512]
                        nc.sync.dma_start(out=dst, in_=st)

            # ---- phase 2: LSTM scan + [w_out, w_ctrl] matvec per step ----
            with tc.tile_pool(name="p2w", bufs=1) as p2w, \
                 tc.tile_pool(name="p2g", bufs=2) as p2g, \
                 tc.tile_pool(name="p2h", bufs=2) as p2h, \
                 tc.tile_pool(name="p2c", bufs=2) as p2c, \
                 tc.tile_pool(name="p2k", bufs=2) as p2k, \
                 tc.tile_pool(name="p2t", bufs=2) as p2t, \
                 tc.tile_pool(name="p2p", bufs=1, space="PSUM") as p2p, \
                 tc.tile_pool(name="p2r", bufs=1, space="PSUM") as p2r, \
                 tc.tile_pool(name="p2q", bufs=2, space="PSUM") as p2q:
                whh_sb = []
                for k in range(KH):
                    wh = p2w.tile([128, FOURH], f16, tag=f"whhh{k}",
                                  name=f"whhh{k}")
                    nc.sync.dma_start(out=wh, in_=whhh_d.ap()[k])
                    wl = p2w.tile([128, FOURH], f16, tag=f"whhl{k}",
                                  name=f"whhl{k}")
                    nc.sync.dma_start(out=wl, in_=whhl_d.ap()[k])
                    whh_sb.append((wh, wl))
                wpr_sb = []
                for k in range(KH):
                    w = p2w.tile([128, 2], f32, tag=f"wpr{k}", name=f"wpr{k}")
                    nc.sync.dma_start(out=w, in_=wpr_d.ap()[k])
                    wpr_sb.append(w)
                ident = p2w.tile([128, 128], f32, tag="ident")
                make_identity(nc, ident)

                def emit_matvec(tv, hT):
                    # z_t = [w_out, w_ctrl] @ h_t ; deferred one step so the
                    # PE runs it behind the next step's main matmuls instead
                    # of on the recurrence critical path.
                    pzv = p2q.tile([2, 64], f32, tag="pzv", name=f"pzv{tv}")
                    for k in range(KH):
                        nc.tensor.matmul(pzv, wpr_sb[k],
                                         hT[:, k * 64:(k + 1) * 64],
                                         start=(k == 0), stop=(k == KH - 1))
                    zz = p2t.tile([2, 64], f32, tag="zz", name=f"zz{tv}")
                    nc.vector.tensor_copy(zz, pzv)
                    nc.sync.dma_start(
                        out=z_d.ap()[:, tv * 64:(tv + 1) * 64], in_=zz)

                hTh_prev = None
                hTl_prev = None
                c_prev = None
                pending = None
                for t in range(T):
                    gxt = p2g.tile([2 * BL, FOURH // 2], f32, tag="gx",
                                   name=f"gx{t}")
                    nc.sync.dma_start(out=gxt, in_=gx[t])
                    c_new = p2c.tile([2 * BL, H // 2], f32, tag="c",
                                     name=f"c{t}")
                    h_new = p2h.tile([2 * BL, H // 2], f32, tag="h",
                                     name=f"h{t}")
                    def emit_mms(pg, g, co, cw, sl):
                        # both h-halves of gate g computed concurrently:
                        # same stationary h^T tile at array col 0 and 64,
                        # streaming the two W chunks, outputs landing in
                        # partitions 0-63 / 64-127 of one PSUM bank.
                        i = 0
                        nmm = 3 * KH
                        for k in range(KH):
                            ks = slice(k * 64, (k + 1) * 64)
                            wh, wl = whh_sb[k]
                            for lhsT, rhs in ((hTh_prev, wh),
                                              (hTl_prev, wh),
                                              (hTh_prev, wl)):
                                first, last = i == 0, i == nmm - 1
                                nc.tensor.matmul(
                                    pg[0:BL, sl],
                                    lhsT[:, ks],
                                    rhs[:, 1024 * g + co:1024 * g + co + cw],
                                    start=first, stop=last,
                                    tile_position=(0, 0))
                                nc.tensor.matmul(
                                    pg[BL:2 * BL, sl], lhsT[:, ks],
                                    rhs[:, 1024 * g + 512 + co:
                                        1024 * g + 512 + co + cw],
                                    start=first, stop=last,
                                    tile_position=(0, 64))
                                i += 1

                    zt = {}
                    for g in range(4):
                        gs = slice(512 * g, 512 * g + 512)
                        z = p2k.tile([2 * BL, 512], f32, tag=f"z{g}",
                                     name=f"z{t}_{g}")
                        zt[g] = z
                        if t == 0:
                            nc.vector.tensor_copy(z, gxt[:, gs])
                            nc.scalar.activation(z, z,
                                                 Tanh if g == 2 else Sig)
                            continue
                        if g < 3:
                            pg = p2p.tile([2 * BL, 512], f32, tag=f"gp{g}",
                                          name=f"gp{t}_{g}")
                            emit_mms(pg, g, 0, 512, slice(0, 512))
                            nc.vector.tensor_tensor(z, pg, gxt[:, gs], add)
                            nc.scalar.activation(z, z,
                                                 Tanh if g == 2 else Sig)
                        else:
                            # o-gate split into two 256-col halves in two
                            # separate PSUM banks so half-a's epilogue
                            # overlaps half-b's matmuls (same-bank PE-W +
                            # DVE-R would otherwise serialize).
                            pga = p2p.tile([2 * BL, 256], f32, tag="gp3a",
                                           name=f"gp{t}_3a")
                            pgb = p2p.tile([2 * BL, 256], f32, tag="gp3b",
                                           name=f"gp{t}_3b")
                            emit_mms(pga, 3, 0, 256, slice(0, 256))
                            emit_mms(pgb, 3, 256, 256, slice(0, 256))
                    if pending is not None:
                        emit_matvec(*pending)
                        pending = None
                    # c = sig(f)*c_prev + sig(i)*tanh(g)  (full folded width)
                    nc.vector.tensor_tensor(zt[2], zt[0], zt[2], mult)
                    if t > 0:
                        nc.vector.tensor_tensor(zt[1], zt[1], c_prev, mult)
                        nc.vector.tensor_tensor(c_new, zt[1], zt[2], add)
                    else:
                        nc.vector.tensor_copy(c_new, zt[2])
                    nc.scalar.activation(zt[0], c_new, Tanh)
                    if t > 0:
                        z3 = zt[3]
                        for pgh, cs3 in ((pga, slice(0, 256)),
                                         (pgb, slice(256, 512))):
                            nc.vector.tensor_tensor(
                                z3[:, cs3], pgh,
                                gxt[:, 1536 + cs3.start:1536 + cs3.stop],
                                add)
                            nc.scalar.activation(z3[:, cs3], z3[:, cs3], Sig)
                            nc.vector.tensor_tensor(
                                h_new[:, cs3], z3[:, cs3], zt[0][:, cs3],
                                mult)
                    else:
                        nc.vector.tensor_tensor(h_new, zt[3], zt[0], mult)
                    # h^T rebuild from the folded layout via regular matmuls:
                    # out = h_slice.T @ ident[:, 64hh:64hh+64] transposes and
                    # selects fold half hh in one op, all operands at
                    # partition base 0 (base-64 stationaries crash walrus).
                    # k-tile k = logical h cols [128k, 128k+128) = fold half
                    # hh=k//4, fold cols 128*(k%4)+.
                    pht = p2r.tile([128, 512], f32, tag="pht", name=f"pht{t}")
                    for k in (0, 1, 4, 5, 2, 3, 6, 7):
                        hh, kk = divmod(k, 4)
                        nc.tensor.matmul(
                            pht[:, k * 64:(k + 1) * 64],
                            h_new[:, kk * 128:(kk + 1) * 128],
                            ident[:, 64 * hh:64 * hh + 64],
                            start=True, stop=True)
                    # fp16 hi/lo casts straight from PSUM so the next step's
                    # matmuls aren't gated on the fp32 copy (matvec-only).
                    hTh = p2t.tile([128, 512], f16, tag="hTh",
                                   name=f"hTh{t}")
                    nc.vector.tensor_copy(hTh, pht)
                    hTl = p2t.tile([128, 512], f16, tag="hTl",
                                   name=f"hTl{t}")
                    nc.vector.tensor_tensor(hTl, pht, hTh, sub)
                    hT_new = p2t.tile([128, 512], f32, tag="hT",
                                      name=f"hT{t}")
                    nc.vector.tensor_copy(hT_new, pht)
                    pending = (t, hT_new)
                    hTh_prev, hTl_prev, c_prev = hTh, hTl, c_new
                if pending is not None:
                    emit_matvec(*pending)

    nc.compile()
    return nc


def _get_nc(T):
    if T not in _compiled:
        _compiled[T] = _build(T)
    return _compiled[T]


def _split16(w):
    """Split fp32 matrix into fp16 hi/lo pair with hi+lo ~= w (21 bits)."""
    wh = w.astype(np.float16)
    wl = (w - wh.astype(np.float32)).astype(np.float16)
    return wh, wl


def _prep_inputs(x, W_ih, W_hh, b_ih, b_hh, W_out, W_ctrl):
    T = x.shape[0]
    wih = np.ascontiguousarray(W_ih.T).reshape(KD, 128, FOURH)
    whh = np.ascontiguousarray(W_hh.T).reshape(KH, 128, FOURH)
    wihh, wihl = _split16(wih)
    whhh, whhl = _split16(whh)
    wpr = np.ascontiguousarray(
        np.stack([W_out[0], W_ctrl[0, :H]], axis=1)).reshape(KH, 128, 2)
    bia = np.ascontiguousarray(b_ih + b_hh)
    in_maps = []
    for r in range(NCORES):
        xt = np.ascontiguousarray(
            x[:, r * BL:(r + 1) * BL, :].transpose(2, 0, 1).reshape(D, T * BL))
        in_maps.append({"xt": xt, "wihh": wihh, "wihl": wihl,
                        "whhh": whhh, "whhl": whhl, "wpr": wpr, "bia": bia})
    return in_maps


def run_device(x, W_ih, W_hh, b_ih, b_hh, W_out, W_ctrl, trace=False):
    """Run the device part; returns Z (2, T, B) fp32 [h@w_out ; h@w_ctrl_h],
    plus the BassKernelResults (for profiling)."""
    from concourse.bass_utils import run_bass_kernel_spmd

    T = x.shape[0]
    nc = _get_nc(T)
    in_maps = _prep_inputs(x, W_ih, W_hh, b_ih, b_hh, W_out, W_ctrl)
    res = run_bass_kernel_spmd(nc, in_maps, list(range(NCORES)), trace=trace)
    Z = np.empty((2, T, B), np.float32)
    for r in range(NCORES):
        Z[:, :, r * BL:(r + 1) * BL] = res.results[r]["z"].reshape(2, T, BL)
    return Z, res


def _postprocess(Z, u, eps, b_out, W_ctrl, b_ctrl):
    T = Z.shape[1]
    e = np.float64(np.float32(eps[0]))
    logits_all = Z[0] + np.float32(b_out[0])            # (T, B) fp32
    wt = np.float64(W_ctrl[0, H])
    bc = np.float64(b_ctrl[0])
    ts_col = np.arange(T, dtype=np.float64)[:, None]
    zc = Z[1].astype(np.float64) + ts_col * wt + bc
    p = 1.0 / (1.0 + np.exp(-zc))
    p = (1.0 - e) * p + e * 0.05
    p = np.where(np.isclose(p, 0.0), p + 1e-6, p)
    a = u.astype(np.float64) < p                        # (T, B) bool
    Bn = Z.shape[2]
    preds = np.zeros(Bn, np.float64)
    hp = np.full(Bn, -1.0, np.float64)
    for t in range(T):
        halt = a[t]
        upd = halt & (preds == 0.0)
        preds = np.where(upd, logits_all[t].astype(np.float64), preds)
        hpu = (hp == -1.0) & halt
        hp = np.where(hpu, float(t), hp)
    final_logits = logits_all[T - 1].astype(np.float64)
    logits_out = np.where(preds == 0.0, final_logits, preds).astype(np.float32)
    hp2 = np.where(hp == -1.0, float(T - 1), hp)
    halting_points = (hp2 + 1.0).astype(np.float32)
    hmean = np.float32(np.mean(1.0 + hp2) / np.float64(T + 1))
    return logits_out, halting_points, hmean


def kernel(x, u, eps, W_ih, W_hh, b_ih, b_hh, W_out, b_out, W_ctrl, b_ctrl,
           W_base, b_base):
    x = np.asarray(x, np.float32)
    u = np.asarray(u, np.float32)
    Z, _ = run_device(x, np.asarray(W_ih, np.float32),
                      np.asarray(W_hh, np.float32),
                      np.asarray(b_ih, np.float32),
                      np.asarray(b_hh, np.float32),
                      np.asarray(W_out, np.float32),
                      np.asarray(W_ctrl, np.float32))
    return _postprocess(Z, u, np.asarray(eps), np.asarray(b_out),
                        np.asarray(W_ctrl), np.asarray(b_ctrl))


# revision 18
# speedup vs baseline: 2.8028x; 1.0426x over previous
"""Trainium2 Bass kernel for the EARLIEST adaptive-halting LSTM.

Shapes (hardcoded from the problem spec):
  x: (T=128, B=512, D=768), u: (T, B), eps: (1,)
  LSTM: H=1024, gates 4H=4096 (torch order i,f,g,o), C=1 output class.

Strategy:
  - Data-parallel over batch: 8 NeuronCores x 64 batch rows each.
  - The halting controller is observational (a/preds/hp never feed back into
    the recurrence), so the device only computes the LSTM scan plus the two
    per-step matvecs h@w_out and h@w_ctrl.  All epsilon-mixing / Bernoulli /
    first-halt logic runs on the host from the tiny (2, T*64) device output.
  - fp32 matmuls run at 4 cycles/row (2 half-speed HW passes, each with its
    own LDWEIGHTS).  Instead, weights and activations are split into fp16
    hi/lo pairs (W = Wh + Wl, h = hh + hl, each fp16 = 10+ mantissa bits, so
    the pair carries ~21 bits) and each matmul is computed as three
    full-rate fp16 passes Wh*hh + Wh*hl + Wl*hh accumulated in fp32 PSUM.
    That is 3 cycles/row of fp16 work vs 4 for native fp32, with ~fp32
    accuracy (validated against the reference: halting decisions exact).
  - Both phases keep the *small* operand stationary and stream the 4H-wide
    weight matrix as the moving operand (N=512 chunks, stream-bound):
      phase 1: stationary X^T blocks (M=128 of T*B), moving W_ih^T -> Gx in
               (t*b, 4H) layout at full rate.
      phase 2: stationary h^T tiles (M=64 batch), moving W_hh^T.  Gates come
               out as (B, 4H); a per-step PE-transpose pass rebuilds h^T.
"""

import sys

sys.path.insert(0, "/opt/trn_rl_repo")

import numpy as np

T_FULL, B, D, H = 128, 512, 768, 1024
NCORES = 8
BL = B // NCORES            # 64 batch rows per core
KD = D // 128               # 6 K-tiles over D
KH = H // 128               # 8 K-tiles over H
FOURH = 4 * H

_compiled = {}


def _build(T):
    import concourse.bass as bass
    import concourse.tile as tile
    from concourse import bacc, mybir
    from concourse.masks import make_identity

    f32 = mybir.dt.float32
    f16 = mybir.dt.float16
    NTB = (T * BL) // 128   # number of 128-row blocks of the T*B axis
    assert (T * BL) % 128 == 0

    nc = bacc.Bacc("TRN2", target_bir_lowering=False, debug=False,
                   num_devices=NCORES)
    xt_d = nc.dram_tensor("xt", [D, T * BL], f32, kind="ExternalInput")
    wihh_d = nc.dram_tensor("wihh", [KD, 128, FOURH], f16,
                            kind="ExternalInput")
    wihl_d = nc.dram_tensor("wihl", [KD, 128, FOURH], f16,
                            kind="ExternalInput")
    whhh_d = nc.dram_tensor("whhh", [KH, 128, FOURH], f16,
                            kind="ExternalInput")
    whhl_d = nc.dram_tensor("whhl", [KH, 128, FOURH], f16,
                            kind="ExternalInput")
    wpr_d = nc.dram_tensor("wpr", [KH, 128, 2], f32, kind="ExternalInput")
    bia_d = nc.dram_tensor("bia", [FOURH], f32, kind="ExternalInput")
    z_d = nc.dram_tensor("z", [2, T * BL], f32, kind="ExternalOutput")

    add = mybir.AluOpType.add
    sub = mybir.AluOpType.subtract
    mult = mybir.AluOpType.mult
    Sig = mybir.ActivationFunctionType.Sigmoid
    Tanh = mybir.ActivationFunctionType.Tanh

    with tile.TileContext(nc) as tc:
        with tc.tile_pool(name="dram", bufs=1, space="DRAM") as dp:
            # "Folded" gates layout: partition = batch + 64*h_half, free =
            # gate*512 + col.  Lets two M=64 matmuls run concurrently in the
            # PE array via column tiling (tile_position col 0 / 64).
            gx = dp.tile([T, 2 * BL, FOURH // 2], f32)

            # ---- phase 1: Gx[t,b,:] = x_t @ W_ih.T + (b_ih + b_hh) ----
            with tc.tile_pool(name="p1w", bufs=1) as p1w, \
                 tc.tile_pool(name="p1x", bufs=18) as p1x, \
                 tc.tile_pool(name="p1s", bufs=6) as p1s, \
                 tc.tile_pool(name="p1p", bufs=8, space="PSUM") as p1p:
                wih_sb = []
                for k in range(KD):
                    wh = p1w.tile([128, FOURH], f16, tag=f"wihh{k}",
                                  name=f"wihh{k}")
                    nc.sync.dma_start(out=wh, in_=wihh_d.ap()[k])
                    wl = p1w.tile([128, FOURH], f16, tag=f"wihl{k}",
                                  name=f"wihl{k}")
                    nc.sync.dma_start(out=wl, in_=wihl_d.ap()[k])
                    wih_sb.append((wh, wl))
                biasb = p1w.tile([128, FOURH], f32, tag="biasb")
                bsrc = bass.AP(tensor=bia_d.ap().tensor, offset=0,
                               ap=[[0, 128], [1, FOURH]])
                nc.sync.dma_start(out=biasb, in_=bsrc)
                for tbi in range(NTB):
                    xst = []
                    for k in range(KD):
                        xx = p1x.tile([128, 128], f32, tag="xst",
                                      name=f"xst{tbi}_{k}")
                        nc.sync.dma_start(
                            out=xx,
                            in_=xt_d.ap()[k * 128:(k + 1) * 128,
                                          tbi * 128:(tbi + 1) * 128])
                        xh = p1x.tile([128, 128], f16, tag="xsth",
                                      name=f"xsth{tbi}_{k}")
                        nc.vector.tensor_copy(xh, xx)
                        xl = p1x.tile([128, 128], f16, tag="xstl",
                                      name=f"xstl{tbi}_{k}")
                        nc.vector.tensor_tensor(xl, xx, xh, sub)
                        xst.append((xh, xl))
                    for c8 in range(8):
                        cs = slice(c8 * 512, (c8 + 1) * 512)
                        ps = p1p.tile([128, 512], f32, tag="ps",
                                      name=f"ps{tbi}_{c8}")
                        nmm = 3 * KD
                        i = 0
                        for k in range(KD):
                            xh, xl = xst[k]
                            wh, wl = wih_sb[k]
                            for lhsT, rhs in ((xh, wh), (xl, wh), (xh, wl)):
                                nc.tensor.matmul(ps, lhsT, rhs[:, cs],
                                                 start=(i == 0),
                                                 stop=(i == nmm - 1))
                                i += 1
                        st = p1s.tile([128, 512], f32, tag="st",
                                      name=f"st{tbi}_{c8}")
                        nc.vector.tensor_tensor(st, ps, biasb[:, cs], add)
                        g8, hh8 = divmod(c8, 2)
                        dst = gx[2 * tbi:2 * tbi + 2,
                                 64 * hh8:64 * hh8 + 64,
                                 512 * g8:512 * g8 + 512]
                        # Gx write-back on the ACT HWDGE queue: keeps the
                        # strided 256KB writes off the nc.sync queue that
                        # feeds the X stationary loads (PE stalled ~46us
                        # every ~8 blocks when both shared one queue).
                        nc.scalar.dma_start(out=dst, in_=st)

            # ---- phase 2: LSTM scan + [w_out, w_ctrl] matvec per step ----
            with tc.tile_pool(name="p2w", bufs=1) as p2w, \
                 tc.tile_pool(name="p2g", bufs=2) as p2g, \
                 tc.tile_pool(name="p2h", bufs=2) as p2h, \
                 tc.tile_pool(name="p2c", bufs=2) as p2c, \
                 tc.tile_pool(name="p2k", bufs=2) as p2k, \
                 tc.tile_pool(name="p2t", bufs=2) as p2t, \
                 tc.tile_pool(name="p2p", bufs=1, space="PSUM") as p2p, \
                 tc.tile_pool(name="p2r", bufs=1, space="PSUM") as p2r, \
                 tc.tile_pool(name="p2q", bufs=2, space="PSUM") as p2q:
                whh_sb = []
                for k in range(KH):
                    wh = p2w.tile([128, FOURH], f16, tag=f"whhh{k}",
                                  name=f"whhh{k}")
                    nc.sync.dma_start(out=wh, in_=whhh_d.ap()[k])
                    wl = p2w.tile([128, FOURH], f16, tag=f"whhl{k}",
                                  name=f"whhl{k}")
                    nc.sync.dma_start(out=wl, in_=whhl_d.ap()[k])
                    whh_sb.append((wh, wl))
                wpr_sb = []
                for k in range(KH):
                    w = p2w.tile([128, 2], f32, tag=f"wpr{k}", name=f"wpr{k}")
                    nc.sync.dma_start(out=w, in_=wpr_d.ap()[k])
                    wpr_sb.append(w)
                ident = p2w.tile([128, 128], f32, tag="ident")
                make_identity(nc, ident)

                def emit_matvec(tv, hT):
                    # z_t = [w_out, w_ctrl] @ h_t ; deferred one step so the
                    # PE runs it behind the next step's main matmuls instead
                    # of on the recurrence critical path.
                    pzv = p2q.tile([2, 64], f32, tag="pzv", name=f"pzv{tv}")
                    for k in range(KH):
                        nc.tensor.matmul(pzv, wpr_sb[k],
                                         hT[:, k * 64:(k + 1) * 64],
                                         start=(k == 0), stop=(k == KH - 1))
                    zz = p2t.tile([2, 64], f32, tag="zz", name=f"zz{tv}")
                    nc.vector.tensor_copy(zz, pzv)
                    nc.sync.dma_start(
                        out=z_d.ap()[:, tv * 64:(tv + 1) * 64], in_=zz)

                hTh_prev = None
                hTl_prev = None
                c_prev = None
                pending = None
                for t in range(T):
                    gxt = p2g.tile([2 * BL, FOURH // 2], f32, tag="gx",
                                   name=f"gx{t}")
                    nc.sync.dma_start(out=gxt, in_=gx[t])
                    c_new = p2c.tile([2 * BL, H // 2], f32, tag="c",
                                     name=f"c{t}")
                    h_new = p2h.tile([2 * BL, H // 2], f32, tag="h",
                                     name=f"h{t}")
                    def emit_mms(pg, g, co, cw, sl):
                        # both h-halves of gate g computed concurrently:
                        # same stationary h^T tile at array col 0 and 64,
                        # streaming the two W chunks, outputs landing in
                        # partitions 0-63 / 64-127 of one PSUM bank.
                        i = 0
                        nmm = 3 * KH
                        for k in range(KH):
                            ks = slice(k * 64, (k + 1) * 64)
                            wh, wl = whh_sb[k]
                            for lhsT, rhs in ((hTh_prev, wh),
                                              (hTl_prev, wh),
                                              (hTh_prev, wl)):
                                first, last = i == 0, i == nmm - 1
                                nc.tensor.matmul(
                                    pg[0:BL, sl],
                                    lhsT[:, ks],
                                    rhs[:, 1024 * g + co:1024 * g + co + cw],
                                    start=first, stop=last,
                                    tile_position=(0, 0))
                                nc.tensor.matmul(
                                    pg[BL:2 * BL, sl], lhsT[:, ks],
                                    rhs[:, 1024 * g + 512 + co:
                                        1024 * g + 512 + co + cw],
                                    start=first, stop=last,
                                    tile_position=(0, 64))
                                i += 1

                    zt = {}
                    for g in range(4):
                        gs = slice(512 * g, 512 * g + 512)
                        z = p2k.tile([2 * BL, 512], f32, tag=f"z{g}",
                                     name=f"z{t}_{g}")
                        zt[g] = z
                        if t == 0:
                            nc.vector.tensor_copy(z, gxt[:, gs])
                            nc.scalar.activation(z, z,
                                                 Tanh if g == 2 else Sig)
                            continue
                        if g < 3:
                            pg = p2p.tile([2 * BL, 512], f32, tag=f"gp{g}",
                                          name=f"gp{t}_{g}")
                            emit_mms(pg, g, 0, 512, slice(0, 512))
                            nc.vector.tensor_tensor(z, pg, gxt[:, gs], add)
                            nc.scalar.activation(z, z,
                                                 Tanh if g == 2 else Sig)
                        else:
                            # o-gate split into two 256-col halves in two
                            # separate PSUM banks so half-a's epilogue
                            # overlaps half-b's matmuls (same-bank PE-W +
                            # DVE-R would otherwise serialize).
                            pga = p2p.tile([2 * BL, 256], f32, tag="gp3a",
                                           name=f"gp{t}_3a")
                            pgb = p2p.tile([2 * BL, 256], f32, tag="gp3b",
                                           name=f"gp{t}_3b")
                            emit_mms(pga, 3, 0, 256, slice(0, 256))
                            emit_mms(pgb, 3, 256, 256, slice(0, 256))
                    if pending is not None:
                        emit_matvec(*pending)
                        pending = None
                    # c = sig(f)*c_prev + sig(i)*tanh(g)  (full folded width)
                    nc.vector.tensor_tensor(zt[2], zt[0], zt[2], mult)
                    if t > 0:
                        nc.vector.tensor_tensor(zt[1], zt[1], c_prev, mult)
                        nc.vector.tensor_tensor(c_new, zt[1], zt[2], add)
                    else:
                        nc.vector.tensor_copy(c_new, zt[2])
                    nc.scalar.activation(zt[0], c_new, Tanh)
                    if t > 0:
                        z3 = zt[3]
                        for pgh, cs3 in ((pga, slice(0, 256)),
                                         (pgb, slice(256, 512))):
                            nc.vector.tensor_tensor(
                                z3[:, cs3], pgh,
                                gxt[:, 1536 + cs3.start:1536 + cs3.stop],
                                add)
                            nc.scalar.activation(z3[:, cs3], z3[:, cs3], Sig)
                            nc.vector.tensor_tensor(
                                h_new[:, cs3], z3[:, cs3], zt[0][:, cs3],
                                mult)
                    else:
                        nc.vector.tensor_tensor(h_new, zt[3], zt[0], mult)
                    # h^T rebuild from the folded layout via regular matmuls:
                    # out = h_slice.T @ ident[:, 64hh:64hh+64] transposes and
                    # selects fold half hh in one op, all operands at
                    # partition base 0 (base-64 stationaries crash walrus).
                    # k-tile k = logical h cols [128k, 128k+128) = fold half
                    # hh=k//4, fold cols 128*(k%4)+.
                    pht = p2r.tile([128, 512], f32, tag="pht", name=f"pht{t}")
                    for k in (0, 1, 4, 5, 2, 3, 6, 7):
                        hh, kk = divmod(k, 4)
                        nc.tensor.matmul(
                            pht[:, k * 64:(k + 1) * 64],
                            h_new[:, kk * 128:(kk + 1) * 128],
                            ident[:, 64 * hh:64 * hh + 64],
                            start=True, stop=True)
                    # fp16 hi/lo casts straight from PSUM so the next step's
                    # matmuls aren't gated on the fp32 copy (matvec-only).
                    hTh = p2t.tile([128, 512], f16, tag="hTh",
                                   name=f"hTh{t}")
                    nc.vector.tensor_copy(hTh, pht)
                    hTl = p2t.tile([128, 512], f16, tag="hTl",
                                   name=f"hTl{t}")
                    nc.vector.tensor_tensor(hTl, pht, hTh, sub)
                    hT_new = p2t.tile([128, 512], f32, tag="hT",
                                      name=f"hT{t}")
                    nc.vector.tensor_copy(hT_new, pht)
                    pending = (t, hT_new)
                    hTh_prev, hTl_prev, c_prev = hTh, hTl, c_new
                if pending is not None:
                    emit_matvec(*pending)

    nc.compile()
    return nc


def _get_nc(T):
    if T not in _compiled:
        _compiled[T] = _build(T)
    return _compiled[T]


def _split16(w):
    """Split fp32 matrix into fp16 hi/lo pair with hi+lo ~= w (21 bits)."""
    wh = w.astype(np.float16)
    wl = (w - wh.astype(np.float32)).astype(np.float16)
    return wh, wl


def _prep_inputs(x, W_ih, W_hh, b_ih, b_hh, W_out, W_ctrl):
    T = x.shape[0]
    wih = np.ascontiguousarray(W_ih.T).reshape(KD, 128, FOURH)
    whh = np.ascontiguousarray(W_hh.T).reshape(KH, 128, FOURH)
    wihh, wihl = _split16(wih)
    whhh, whhl = _split16(whh)
    wpr = np.ascontiguousarray(
        np.stack([W_out[0], W_ctrl[0, :H]], axis=1)).reshape(KH, 128, 2)
    bia = np.ascontiguousarray(b_ih + b_hh)
    in_maps = []
    for r in range(NCORES):
        xt = np.ascontiguousarray(
            x[:, r * BL:(r + 1) * BL, :].transpose(2, 0, 1).reshape(D, T * BL))
        in_maps.append({"xt": xt, "wihh": wihh, "wihl": wihl,
                        "whhh": whhh, "whhl": whhl, "wpr": wpr, "bia": bia})
    return in_maps


def run_device(x, W_ih, W_hh, b_ih, b_hh, W_out, W_ctrl, trace=False):
    """Run the device part; returns Z (2, T, B) fp32 [h@w_out ; h@w_ctrl_h],
    plus the BassKernelResults (for profiling)."""
    from concourse.bass_utils import run_bass_kernel_spmd

    T = x.shape[0]
    nc = _get_nc(T)
    in_maps = _prep_inputs(x, W_ih, W_hh, b_ih, b_hh, W_out, W_ctrl)
    res = run_bass_kernel_spmd(nc, in_maps, list(range(NCORES)), trace=trace)
    Z = np.empty((2, T, B), np.float32)
    for r in range(NCORES):
        Z[:, :, r * BL:(r + 1) * BL] = res.results[r]["z"].reshape(2, T, BL)
    return Z, res


def _postprocess(Z, u, eps, b_out, W_ctrl, b_ctrl):
    T = Z.shape[1]
    e = np.float64(np.float32(eps[0]))
    logits_all = Z[0] + np.float32(b_out[0])            # (T, B) fp32
    wt = np.float64(W_ctrl[0, H])
    bc = np.float64(b_ctrl[0])
    ts_col = np.arange(T, dtype=np.float64)[:, None]
    zc = Z[1].astype(np.float64) + ts_col * wt + bc
    p = 1.0 / (1.0 + np.exp(-zc))
    p = (1.0 - e) * p + e * 0.05
    p = np.where(np.isclose(p, 0.0), p + 1e-6, p)
    a = u.astype(np.float64) < p                        # (T, B) bool
    Bn = Z.shape[2]
    preds = np.zeros(Bn, np.float64)
    hp = np.full(Bn, -1.0, np.float64)
    for t in range(T):
        halt = a[t]
        upd = halt & (preds == 0.0)
        preds = np.where(upd, logits_all[t].astype(np.float64), preds)
        hpu = (hp == -1.0) & halt
        hp = np.where(hpu, float(t), hp)
    final_logits = logits_all[T - 1].astype(np.float64)
    logits_out = np.where(preds == 0.0, final_logits, preds).astype(np.float32)
    hp2 = np.where(hp == -1.0, float(T - 1), hp)
    halting_points = (hp2 + 1.0).astype(np.float32)
    hmean = np.float32(np.mean(1.0 + hp2) / np.float64(T + 1))
    return logits_out, halting_points, hmean


def kernel(x, u, eps, W_ih, W_hh, b_ih, b_hh, W_out, b_out, W_ctrl, b_ctrl,
           W_base, b_base):
    x = np.asarray(x, np.float32)
    u = np.asarray(u, np.float32)
    Z, _ = run_device(x, np.asarray(W_ih, np.float32),
                      np.asarray(W_hh, np.float32),
                      np.asarray(b_ih, np.float32),
                      np.asarray(b_hh, np.float32),
                      np.asarray(W_out, np.float32),
                      np.asarray(W_ctrl, np.float32))
    return _postprocess(Z, u, np.asarray(eps), np.asarray(b_out),
                        np.asarray(W_ctrl), np.asarray(b_ctrl))
